# revision 1
# baseline (speedup 1.0000x reference)
"""GCN2 (GCNII) message-passing kernel for 8 Trainium2 NeuronCores.

Strategy (1D node sharding per the spec sharding_hint), v2 "transposed"
pipeline:
- Nodes padded to NPAD = 8*NBLK*128 and sharded contiguously across 8 cores.
- Per-node-feature state is kept feature-major: hT/h0T/zT are [64, shard]
  SBUF tiles, so BatchNorm affine + ReLU + bias become per-partition
  scale/bias ops on the (otherwise idle) Activation engine, and BN stats
  come for free from activation accum_out columns.
- Edges (self-loops excluded) are partitioned by destination core, sorted
  by (chunk of CHUNK_BLKS dest blocks, q=rr%4, dest block), padded to 128 per
  (chunk, q) bucket uniformly across cores (SPMD).
- The support table [NPAD, 64] f16 lives in DRAM with the partition-major
  row remap rr = c*shard + p*nblk + b; one dma_gather per chunk fetches a
  512-byte element (4 consecutive table rows, idx w = rr//4, int16) per
  edge; q selects the 64-col quarter at matmul time.
- Segment-sum on the PE: per 128-edge tile, DVE builds a one-hot
  P[e, d] = (iota==col_rel)*norm and PE accumulates
  aggT[feat, dest] += g_slice.T @ P into the dest block's [64,128] PSUM
  tile.  The same PSUM tile also accumulates initial = (w2+aI).T @ h0T
  and the self-loop term (dinv^2-scaled local support via identity
  matmul), so z = agg + initial never materializes through DVE.
- BN stats per core ride AllGather (cheaper than AllReduce); the support
  shards are AllGathered into the next layer's DRAM table.
- f16 table/gather/P/weights (validated ~7e-4 rel err); f32 PSUM/stats.
"""
import math
from contextlib import ExitStack

import numpy as np

import concourse.bass as bass
import concourse.bacc as bacc
import concourse.tile as tile
from concourse import mybir
from concourse.bass_utils import run_bass_kernel_spmd
from concourse.masks import make_identity

DT = mybir.dt
F16 = DT.float16
F32 = DT.float32
AF = mybir.ActivationFunctionType
OP = mybir.AluOpType

NC = 8
BLK = 128
ALPHA = 0.5
EPS = 1e-5
CHUNK_BLKS = 3
NVAR = 4           # max dest-block span of one edge tile
GRP = 4            # blocks per 512-col matmul group
DEBUG = False
BUILD_ONLY = False
TIME_REPEATS = 0
EXEC_NS = None
EXEC_TIMES = None
TRACE = False
TRACE_DIR = None
LAST = None


# ---------------------------------------------------------------- host prep
def _host_prep(edge_index, n_nodes, npad, shard, nblk, chunk_blks):
    """Per-core edge arrays + SPMD-uniform gather/matmul schedule.

    Table rows use the partition-major remap rr = c*shard + p*nblk + b.
    dma_gather (int16 idx, 512B elems) reads the [npad//4, 4*64] wide view:
    idx w = rr//4 selects a 4-node group; q = rr%4 picks the 64-col quarter,
    chosen per tile (edges sorted by (chunk, q, block)).
    """
    e = np.asarray(edge_index)
    row = e[0].astype(np.int64)
    col = e[1].astype(np.int64)
    # degrees include the self loop (gcn_norm adds one per node)
    deg = np.bincount(col, minlength=n_nodes).astype(np.float64) + 1.0
    dinv = deg ** -0.5
    norm = (dinv[row] * dinv[col]).astype(np.float32)
    selfn_full = np.zeros(npad, np.float32)
    selfn_full[:n_nodes] = (dinv * dinv).astype(np.float32)

    core = col // shard
    blk = (col % shard) // BLK
    crel_g = col % shard

    c_src = row // shard
    r_src = row % shard
    rr = c_src * shard + (r_src % BLK) * nblk + (r_src // BLK)
    w_all = rr // 4
    q_all = rr % 4

    chunks_b = []
    b0 = 0
    while b0 < nblk:
        chunks_b.append((b0, min(b0 + chunk_blks, nblk)))
        b0 = min(b0 + chunk_blks, nblk)
    nchunk = len(chunks_b)
    chunk_of = np.zeros(nblk, dtype=np.int64)
    for ci, (cb0, cb1) in enumerate(chunks_b):
        chunk_of[cb0:cb1] = ci
    key_chunk = chunk_of[blk]

    order = np.lexsort((blk, q_all, key_chunk, core))
    S = dict(w=w_all[order], q=q_all[order], blk=blk[order], core=core[order],
             chunk=key_chunk[order], crel=crel_g[order], nrm=norm[order])

    cnt = np.zeros((NC, nchunk, 4), dtype=np.int64)
    np.add.at(cnt, (S["core"], S["chunk"], S["q"]), 1)
    run_len = (-(-cnt.max(axis=0) // BLK) * BLK)        # [nchunk, 4]
    run_len = np.maximum(run_len, BLK)

    ntiles = int(run_len.sum()) // BLK
    tot = ntiles * BLK
    p_w = np.zeros((NC, tot), dtype=np.int16)
    p_crel = np.full((NC, tot), 20000.0, dtype=np.float32)
    p_nrm = np.zeros((NC, tot), dtype=np.float32)
    p_blk = np.full((NC, tot), -1, dtype=np.int64)

    run_off = np.zeros((nchunk, 4), dtype=np.int64)
    acc = 0
    for ci in range(nchunk):
        for q in range(4):
            run_off[ci, q] = acc
            acc += run_len[ci, q]

    grp_key = S["core"] * (nchunk * 4) + S["chunk"] * 4 + S["q"]
    grp_change = np.concatenate([[True], grp_key[1:] != grp_key[:-1]])
    grp_start = np.where(grp_change)[0]
    rank = np.arange(len(grp_key)) - np.repeat(
        grp_start, np.diff(np.concatenate([grp_start, [len(grp_key)]])))
    slot = run_off[S["chunk"], S["q"]] + rank
    p_w[S["core"], slot] = S["w"].astype(np.int16)
    p_crel[S["core"], slot] = S["crel"].astype(np.float32)
    p_nrm[S["core"], slot] = S["nrm"]
    p_blk[S["core"], slot] = S["blk"]

    # schedule: per chunk -> one call + tiles
    chunks = []
    gidx_cols = 0
    tile_global = 0
    first_tile = {}
    last_tile = {}
    for ci, (cb0, cb1) in enumerate(chunks_b):
        nidx = int(run_len[ci].sum())
        calls = []
        tiles = []
        erel = 0
        for q in range(4):
            off = int(run_off[ci, q])
            nq = int(run_len[ci, q])
            calls.append(dict(gi0=gidx_cols, ncols16=nq // 16, num_idxs=nq,
                              base=off, e0=erel))
            gidx_cols += nq // 16
            nt = nq // BLK
            for t in range(nt):
                s0 = off + t * BLK
                blks = p_blk[:, s0:s0 + BLK]
                real = blks >= 0
                if real.any():
                    b_lo = int(blks[real].min())
                    b_hi = int(blks[real].max())
                else:
                    b_lo = b_hi = cb0
                assert b_hi - b_lo < NVAR, "tile spans too many blocks"
                td = dict(slot0=s0, erel=erel, q=q, b_lo=b_lo,
                          tid=tile_global,
                          pairs=list(range(b_lo, b_hi + 1)))
                for b in td["pairs"]:
                    if b not in first_tile:
                        first_tile[b] = tile_global
                    last_tile[b] = tile_global
                tiles.append(td)
                tile_global += 1
                erel += 1
        chunks.append(dict(calls=calls, tiles=tiles, b0=cb0, b1=cb1,
                           nelem=nidx))
    assert tile_global == ntiles

    for ch in chunks:
        for td in ch["tiles"]:
            td["flags"] = [(b, b - td["b_lo"],
                            first_tile[b] == td["tid"],
                            last_tile[b] == td["tid"]) for b in td["pairs"]]

    # edat: col_rel - b_lo*128 (f32); enrm: norm (f16) per tile slot
    edat = np.zeros((NC, BLK, ntiles), dtype=np.float32)
    enrm = np.zeros((NC, BLK, ntiles), dtype=np.float32)
    ti = 0
    for ch in chunks:
        for td in ch["tiles"]:
            s0 = td["slot0"]
            cr = p_crel[:, s0:s0 + BLK] - (td["b_lo"] * BLK)
            cr[p_blk[:, s0:s0 + BLK] < 0] = 20000.0
            edat[:, :, ti] = cr
            enrm[:, :, ti] = p_nrm[:, s0:s0 + BLK]
            ti += 1
    assert ti == ntiles

    # gidx: per call, 16-wrapped layout replicated across 128 partitions
    gidx = np.zeros((NC, BLK, gidx_cols), dtype=np.int16)
    for ch in chunks:
        for ca in ch["calls"]:
            base, n = ca["base"], ca["num_idxs"]
            vals = p_w[:, base:base + n]
            wrap = vals.reshape(NC, n // 16, 16).transpose(0, 2, 1)
            gi0 = ca["gi0"]
            for rep in range(8):
                gidx[:, rep * 16:(rep + 1) * 16, gi0:gi0 + n // 16] = wrap

    # per-core dinv^2 column layout [128, nblk]
    selfn = np.zeros((NC, BLK, nblk), dtype=np.float32)
    for c in range(NC):
        sl = selfn_full[c * shard:(c + 1) * shard].reshape(nblk, BLK)
        selfn[c] = sl.T
    return dict(gidx=gidx, edat=edat, enrm=enrm, chunks=chunks,
                ntiles=ntiles, gcols=gidx_cols, selfn=selfn)


# ---------------------------------------------------------------- program
def _build(nc, cfg):
    fin = cfg["fin"]
    hid = cfg["hid"]
    outd = cfg["outd"]
    nlay = cfg["nlay"]
    shard = cfg["shard"]
    nblk = cfg["nblk"]
    npad = cfg["npad"]
    n_nodes = cfg["n"]
    ntiles = cfg["ntiles"]

    xT = nc.declare_dram_parameter("xT", [fin, shard], F16, isOutput=False)
    gidx = nc.declare_dram_parameter("gidx", [BLK, cfg["gcols"]], DT.int16, isOutput=False)
    edat = nc.declare_dram_parameter("edat", [BLK, ntiles], F32, isOutput=False)
    enrm = nc.declare_dram_parameter("enrm", [BLK, ntiles], F32, isOutput=False)
    wi = nc.declare_dram_parameter("wi", [fin, hid], F16, isOutput=False)
    w1p = nc.declare_dram_parameter("w1p", [nlay, hid, hid], F16, isOutput=False)
    w2p = nc.declare_dram_parameter("w2p", [nlay, hid, hid], F16, isOutput=False)
    gb = nc.declare_dram_parameter("gb", [hid, 2 * nlay], F32, isOutput=False)
    wo = nc.declare_dram_parameter("wo", [hid, outd], F16, isOutput=False)
    bvec = nc.declare_dram_parameter("bvec", [hid, 2], F32, isOutput=False)
    selfn = nc.declare_dram_parameter("selfn", [BLK, nblk], F32, isOutput=False)
    out_p = nc.declare_dram_parameter("out", [outd, shard], F16, isOutput=True)
    dbg = cfg.get("debug", False)
    if dbg:
        dbg_h = nc.declare_dram_parameter("dbg_h", [hid, shard], F16, isOutput=True)
        dbg_sup = nc.declare_dram_parameter("dbg_sup", [BLK, nblk * hid], F16, isOutput=True)
        dbg_self = nc.declare_dram_parameter("dbg_self", [BLK, nblk * hid], F16, isOutput=True)
        dbg_z = nc.declare_dram_parameter("dbg_z", [hid, shard], F16, isOutput=True)
        dbg_st = nc.declare_dram_parameter("dbg_st", [hid, 2], F32, isOutput=True)
        dbg_sc = nc.declare_dram_parameter("dbg_sc", [hid, 2], F32, isOutput=True)

    core_ids = list(range(NC))
    inv_n = 1.0 / float(n_nodes)

    # node-column groups of GRP blocks (512 cols) for wide matmuls
    groups = []
    b0 = 0
    while b0 < nblk:
        b1 = min(b0 + GRP, nblk)
        groups.append((b0, b1))
        b0 = b1

    with tile.TileContext(nc) as tc, ExitStack() as ctx:
        const = ctx.enter_context(tc.tile_pool(name="const", bufs=1))
        dram = ctx.enter_context(tc.tile_pool(name="dram", bufs=1, space="DRAM"))

        tables = [dram.tile([npad, hid], F16, addr_space="Shared",
                            name=f"table{i}") for i in range(nlay)]
        sup_local = dram.tile([shard, hid], F16)
        stats_in = dram.tile([hid, 2], F32)
        stats_outs = [dram.tile([NC * hid, 2], F32, addr_space="Shared",
                                name=f"statso{i}") for i in range(nlay)]

        # ---- constants
        iotas = []
        for v in range(NVAR):
            iota_i = const.tile([BLK, BLK], DT.int16, tag="ioti")
            nc.gpsimd.iota(iota_i, pattern=[[1, BLK]], base=v * BLK,
                           channel_multiplier=0)
            iota_v = const.tile([BLK, BLK], F16, tag=f"iotf{v}")
            nc.vector.tensor_copy(iota_v, iota_i)
            iotas.append(iota_v)
        ident16 = const.tile([BLK, BLK], F16)
        make_identity(nc, ident16)

        wi_sb = const.tile([fin, hid], F16)
        nc.sync.dma_start(out=wi_sb, in_=wi[:, :])
        w1_sb = const.tile([hid, nlay * hid], F16)
        w2_sb = const.tile([hid, nlay * hid], F16)
        for l in range(nlay):
            nc.sync.dma_start(out=w1_sb[:, l * hid:(l + 1) * hid], in_=w1p[l, :, :])
            nc.sync.dma_start(out=w2_sb[:, l * hid:(l + 1) * hid], in_=w2p[l, :, :])
        wo_sb = const.tile([hid, outd], F16)
        nc.sync.dma_start(out=wo_sb, in_=wo[:, :])
        gb_sb = const.tile([hid, 2 * nlay], F32)
        nc.sync.dma_start(out=gb_sb, in_=gb[:, :])
        bvec_sb = const.tile([hid, 2], F32)
        nc.sync.dma_start(out=bvec_sb, in_=bvec[:, :])
        selfn_sb = const.tile([BLK, nblk], F32)
        nc.sync.dma_start(out=selfn_sb, in_=selfn[:, :])
        gidx_sb = const.tile([BLK, cfg["gcols"]], DT.int16)
        edat_sb = const.tile([BLK, ntiles], F32)
        enrm_sb = const.tile([BLK, ntiles], F32)

        # ---- persistent state (feature-major)
        hT = const.tile([hid, shard], F16)
        h0T = const.tile([hid, shard], F16)
        zT = const.tile([hid, shard], F16)
        supsend = const.tile([BLK, nblk * hid], F16)
        stats_pg = const.tile([hid, 2 * nblk], F32)
        stats_col = const.tile([hid, 2], F32)
        statsg_sb = const.tile([hid, 2 * NC], F32)

        gpool = ctx.enter_context(tc.tile_pool(name="gpool", bufs=3))
        ppool = ctx.enter_context(tc.tile_pool(name="ppool", bufs=8))
        tpool = ctx.enter_context(tc.tile_pool(name="tpool", bufs=5))
        spool = ctx.enter_context(tc.tile_pool(name="spool", bufs=4))

        # ---- input layer emitted per-group inside layer 0's support loop

        # edge-schedule tables aren't needed until the first gather;
        # loading them here overlaps the input layer's compute
        nc.sync.dma_start(out=gidx_sb, in_=gidx[:, :])
        nc.sync.dma_start(out=edat_sb, in_=edat[:, :])
        nc.sync.dma_start(out=enrm_sb, in_=enrm[:, :])

        if dbg:
            nc.sync.dma_start(out=dbg_h[:, :], in_=hT)

        # ---- layers
        scl_p = sht_p = None
        for l in range(nlay):
            w1s = w1_sb[:, l * hid:(l + 1) * hid]
            w2s = w2_sb[:, l * hid:(l + 1) * hid]

            # (a) fused: hT += relu(scl*zT+sht) [layer l-1 BN], then
            #     supT = (w1+I).T @ hT; transpose to node-major; write table
            #     shard + dinv^2-scaled self-loop copy.  Copies on DVE (wide).
            lay_in = ExitStack()
            sup_ps = lay_in.enter_context(
                tc.tile_pool(name=f"sup{l}", bufs=3, space="PSUM"))
            tr_ps = lay_in.enter_context(
                tc.tile_pool(name=f"tr{l}", bufs=3, space="PSUM"))
            if l == 0:
                xpool = lay_in.enter_context(tc.tile_pool(name="xpool", bufs=3))
                in_ps = lay_in.enter_context(
                    tc.tile_pool(name="in_ps", bufs=2, space="PSUM"))
            if True:
                for (g0, g1) in groups:
                    w = (g1 - g0) * BLK
                    c0 = g0 * BLK
                    if l == 0:
                        xg = xpool.tile([fin, GRP * BLK], F16, tag="xg")
                        nc.sync.dma_start(out=xg[:, :w], in_=xT[:, c0:c0 + w])
                        ph = in_ps.tile([hid, GRP * BLK], F32, tag="ph")
                        nc.tensor.matmul(ph[:, :w], lhsT=wi_sb, rhs=xg[:, :w],
                                         start=True, stop=True)
                        nc.scalar.activation(out=hT[:, c0:c0 + w],
                                             in_=ph[:, :w], func=AF.Relu,
                                             bias=bvec_sb[:, 0:1], scale=1.0)
                        nc.vector.tensor_copy(h0T[:, c0:c0 + w],
                                              hT[:, c0:c0 + w])
                    if scl_p is not None:
                        rt = tpool.tile([hid, GRP * BLK], F16, tag="rt")
                        nc.scalar.activation(out=rt[:, :w], in_=zT[:, c0:c0 + w],
                                             func=AF.Relu, scale=scl_p[:, 0:1],
                                             bias=sht_p[:, 0:1])
                        nc.gpsimd.tensor_tensor(out=hT[:, c0:c0 + w],
                                                in0=hT[:, c0:c0 + w],
                                                in1=rt[:, :w], op=OP.add)
                    sp = sup_ps.tile([hid, GRP * BLK], F32, tag="sp")
                    nc.tensor.matmul(sp[:, :w], lhsT=w1s, rhs=hT[:, c0:c0 + w],
                                     start=True, stop=True)
                    spf = spool.tile([hid, GRP * BLK], F16, tag="spf")
                    if groups.index((g0, g1)) % 4 == 3:
                        nc.scalar.activation(out=spf[:, :w], in_=sp[:, :w],
                                             func=AF.Copy)
                    else:
                        nc.vector.tensor_copy(spf[:, :w], sp[:, :w])
                    tp = tr_ps.tile([BLK, GRP * hid], F16, tag="tp")
                    for b in range(g0, g1):
                        boff = (b - g0) * BLK
                        toff = (b - g0) * hid
                        nc.tensor.transpose(out=tp[:, toff:toff + hid],
                                            in_=spf[:, boff:boff + BLK],
                                            identity=ident16[:hid, :hid])
                    nc.vector.tensor_copy(
                        supsend[:, g0 * hid:g1 * hid],
                        tp[:, :(g1 - g0) * hid])
                    gi = groups.index((g0, g1))
                    if gi % 5 == 4 or g1 == nblk:
                        s0 = groups[gi - gi % 5][0]
                        nc.sync.dma_start(
                            out=sup_local[:, :].rearrange(
                                "(p b) f -> p (b f)", p=BLK)[:, s0 * hid:g1 * hid],
                            in_=supsend[:, s0 * hid:g1 * hid])
            lay_in.close()
            if dbg and l == 0:
                nc.sync.dma_start(out=dbg_sup[:, :], in_=supsend)
            nc.gpsimd.collective_compute(
                "AllGather", OP.bypass, replica_groups=[core_ids],
                ins=[sup_local.opt()], outs=[tables[l].opt()])

            # (b) edge aggregation: one [64, 4*128] PSUM bank per chunk
            lay_ctx = ExitStack()
            agg_ps = lay_ctx.enter_context(
                tc.tile_pool(name=f"agg{l}", bufs=2 * CHUNK_BLKS,
                             space="PSUM"))
            tw = tables[l][:, :].rearrange("(w q) f -> w (q f)", q=4)
            for ci, ch in enumerate(cfg["chunks"]):
                nel = ch["nelem"]
                cb0, cb1 = ch["b0"], ch["b1"]
                g = gpool.tile([BLK, cfg["max_chunk_elems"] * 2], F16,
                               tag="g")
                ca0 = ch["calls"][0]
                nc.gpsimd.dma_gather(
                    out_ap=g[:, :nel * 2].rearrange(
                        "p (t f) -> p t f", f=256),
                    in_ap=tw[:, :],
                    idxs_ap=gidx_sb[:, ca0["gi0"]:ca0["gi0"] + nel // 16],
                    num_idxs=nel,
                    num_idxs_reg=nel,
                    elem_size=256,
                    elem_step=256,
                    single_packet=(nel <= 1024),
                    queue_num=ci % 2,
                )
                aggt = {}
                for b in range(cb0, cb1):
                    aggt[b] = agg_ps.tile([hid, BLK], F32, tag="agg",
                                          name=f"agg_b{b}")
                    # initial = (w2+aI).T @ h0T block
                    nc.tensor.matmul(aggt[b], lhsT=w2s,
                                     rhs=h0T[:, b * BLK:(b + 1) * BLK],
                                     start=True, stop=False)
                    # self loop: local support against dinv^2-scaled diag
                    dg = ppool.tile([BLK, BLK], F16, tag="dg")
                    nc.vector.tensor_scalar(out=dg, in0=ident16,
                                            scalar1=selfn_sb[:, b:b + 1],
                                            scalar2=None, op0=OP.mult)
                    nc.tensor.matmul(aggt[b],
                                     lhsT=supsend[:, b * hid:(b + 1) * hid],
                                     rhs=dg,
                                     start=False, stop=False)
                for td in ch["tiles"]:
                    ti = td["tid"]
                    gsl = g[:, td["erel"] * 256 + td["q"] * hid:
                            td["erel"] * 256 + td["q"] * hid + hid]
                    for (b, var, st, sp) in td["flags"]:
                        pt = ppool.tile([BLK, BLK], F16, tag="p")
                        nc.vector.tensor_scalar(
                            out=pt, in0=iotas[var],
                            scalar1=edat_sb[:, ti:ti + 1],
                            scalar2=enrm_sb[:, ti:ti + 1],
                            op0=OP.is_equal, op1=OP.mult)
                        nc.tensor.matmul(aggt[b], lhsT=gsl,
                                         rhs=pt, start=False, stop=sp)
                for b in range(cb0, cb1):
                    # evict: z to SBUF (f16) + stats accums
                    nc.scalar.activation(
                        out=zT[:, b * BLK:(b + 1) * BLK],
                        in_=aggt[b], func=AF.Copy,
                        accum_out=stats_pg[:, b:b + 1])
                    zq = tpool.tile([hid, BLK], F32, tag="zq")
                    nc.scalar.activation(
                        out=zq, in_=aggt[b], func=AF.Square,
                        accum_out=stats_pg[:, nblk + b:nblk + b + 1])
            lay_ctx.close()

            if dbg and l == 0:
                nc.sync.dma_start(out=dbg_z[:, :], in_=zT)

            # (c) stats: local reduce + AllGather + global reduce
            scr = tpool.tile([hid, nblk], F32, tag="scr")
            nc.scalar.activation(out=scr, in_=stats_pg[:, :nblk], func=AF.Copy,
                                 accum_out=stats_col[:, 0:1])
            scr2 = tpool.tile([hid, nblk], F32, tag="scr2")
            nc.scalar.activation(out=scr2, in_=stats_pg[:, nblk:], func=AF.Copy,
                                 accum_out=stats_col[:, 1:2])
            nc.sync.dma_start(out=stats_in.opt(), in_=stats_col)
            nc.gpsimd.collective_compute(
                "AllGather", OP.bypass, replica_groups=[core_ids],
                ins=[stats_in.opt()], outs=[stats_outs[l].opt()])
            nc.sync.dma_start(
                out=statsg_sb[:, :].rearrange("p (c s) -> p c s", c=NC),
                in_=stats_outs[l][:, :].rearrange("(c p) s -> p c s", c=NC))
            sumz = tpool.tile([hid, 1], F32, tag="sumz")
            sumq = tpool.tile([hid, 1], F32, tag="sumq")
            dscr = tpool.tile([hid, 2 * NC], F32, tag="dscr")
            nc.scalar.activation(
                out=dscr[:, :NC],
                in_=statsg_sb[:, :].rearrange("p (c s) -> p s c", c=NC)[:, 0, :],
                func=AF.Copy, accum_out=sumz)
            nc.scalar.activation(
                out=dscr[:, NC:],
                in_=statsg_sb[:, :].rearrange("p (c s) -> p s c", c=NC)[:, 1, :],
                func=AF.Copy, accum_out=sumq)

            # (d) BN affine params (all [64,1] columns)
            mt = tpool.tile([hid, 1], F32, tag="mt")
            nc.vector.tensor_scalar(out=mt, in0=sumz, scalar1=inv_n,
                                    scalar2=None, op0=OP.mult)
            vt = tpool.tile([hid, 1], F32, tag="vt")
            nc.vector.tensor_scalar(out=vt, in0=sumq, scalar1=inv_n,
                                    scalar2=None, op0=OP.mult)
            mm = tpool.tile([hid, 1], F32, tag="mm")
            nc.vector.tensor_tensor(out=mm, in0=mt, in1=mt, op=OP.mult)
            nc.vector.tensor_tensor(out=vt, in0=vt, in1=mm, op=OP.subtract)
            nc.vector.tensor_scalar(out=vt, in0=vt, scalar1=EPS, scalar2=None,
                                    op0=OP.add)
            rstd = tpool.tile([hid, 1], F32, tag="rstd")
            nc.vector.reciprocal(rstd, vt)
            nc.scalar.activation(out=rstd, in_=rstd, func=AF.Sqrt)
            scl = const.tile([hid, 1], F32, tag=f"scl{l}")
            nc.vector.tensor_tensor(out=scl, in0=gb_sb[:, 2 * l:2 * l + 1],
                                    in1=rstd, op=OP.mult)
            sht = const.tile([hid, 1], F32, tag=f"sht{l}")
            nc.vector.tensor_tensor(out=sht, in0=mt, in1=scl, op=OP.mult)
            nc.vector.tensor_tensor(out=sht, in0=gb_sb[:, 2 * l + 1:2 * l + 2],
                                    in1=sht, op=OP.subtract)
            if dbg and l == 0:
                nc.sync.dma_start(out=dbg_st[:, :], in_=stats_col)
                dbg_sc_t = const.tile([hid, 2], F32, tag="dbg_sc_t")
                nc.vector.tensor_copy(dbg_sc_t[:, 0:1], scl)
                nc.vector.tensor_copy(dbg_sc_t[:, 1:2], sht)
                nc.sync.dma_start(out=dbg_sc[:, :], in_=dbg_sc_t)
            scl_p, sht_p = scl, sht

        # ---- output layer: hT += relu(scl*zT+sht); outT = Wo.T @ hT + bo
        with tc.tile_pool(name="out_ps", bufs=2, space="PSUM") as out_ps:
            for (g0, g1) in groups:
                w = (g1 - g0) * BLK
                c0 = g0 * BLK
                rt = tpool.tile([hid, GRP * BLK], F16, tag="rt")
                nc.scalar.activation(out=rt[:, :w], in_=zT[:, c0:c0 + w],
                                     func=AF.Relu, scale=scl_p[:, 0:1],
                                     bias=sht_p[:, 0:1])
                nc.vector.tensor_tensor(out=hT[:, c0:c0 + w],
                                        in0=hT[:, c0:c0 + w], in1=rt[:, :w],
                                        op=OP.add)
                po = out_ps.tile([outd, GRP * BLK], F32, tag="po")
                nc.tensor.matmul(po[:, :w], lhsT=wo_sb, rhs=hT[:, c0:c0 + w],
                                 start=True, stop=True)
                ost = tpool.tile([outd, GRP * BLK], F16, tag="ost")
                nc.vector.tensor_copy(ost[:, :w], po[:, :w])
                nc.sync.dma_start(out=out_p[:, c0:c0 + w], in_=ost[:, :w])
    return nc


# ------------------------------------------------------------- timed runner
def _run_spmd_timed(nc, in_maps, n_cores, repeats):
    """Mirror of bass2jax.run_bass_via_pjrt with repeat timing (no donation,
    inputs pre-staged on device)."""
    import jax
    import time
    from jax.sharding import Mesh, PartitionSpec, NamedSharding
    from jax.experimental.shard_map import shard_map
    from concourse import bass2jax

    bass2jax.install_neuronx_cc_hook()
    partition_name = (nc.partition_id_tensor.name
                     if nc.partition_id_tensor else None)
    in_names, out_names, out_avals, zero_outs = [], [], [], []
    for alloc in nc.m.functions[0].allocations:
        if not isinstance(alloc, mybir.MemoryLocationSet):
            continue
        name = alloc.memorylocations[0].name
        if alloc.kind == "ExternalInput":
            if name != partition_name:
                in_names.append(name)
        elif alloc.kind == "ExternalOutput":
            shape = tuple(alloc.tensor_shape)
            dtype = mybir.dt.np(alloc.dtype)
            out_names.append(name)
            out_avals.append(jax.core.ShapedArray(shape, dtype))
            zero_outs.append(np.zeros(shape, dtype))
    n_params = len(in_names)
    in_names_full = list(in_names) + list(out_names)
    if partition_name is not None:
        in_names_full.append(partition_name)

    def _body(*args):
        operands = list(args)
        if partition_name is not None:
            operands.append(bass2jax.partition_id_tensor())
        outs = bass2jax._bass_exec_p.bind(
            *operands, out_avals=tuple(out_avals),
            in_names=tuple(in_names_full), out_names=tuple(out_names),
            lowering_input_output_aliases=(),
            sim_require_finite=True, sim_require_nnan=True, nc=nc)
        return tuple(outs)

    devices = jax.devices()[:n_cores]
    mesh = Mesh(np.asarray(devices), ("core",))
    spec = PartitionSpec("core")
    n_outs = len(out_avals)
    sharded = jax.jit(shard_map(
        _body, mesh=mesh, in_specs=(spec,) * (n_params + n_outs),
        out_specs=(spec,) * n_outs, check_rep=False), keep_unused=True)
    concat_in = [
        np.concatenate([np.asarray(in_maps[c][nm]) for c in range(n_cores)],
                       axis=0)
        for nm in in_names]
    concat_zeros = [np.zeros((n_cores * z.shape[0], *z.shape[1:]), z.dtype)
                    for z in zero_outs]
    sh = NamedSharding(mesh, spec)
    dev_in = [jax.device_put(a, sh) for a in concat_in + concat_zeros]
    for a in dev_in:
        a.block_until_ready()
    out_arrs = sharded(*dev_in)
    jax.block_until_ready(out_arrs)
    times = []
    for _ in range(repeats):
        t0 = time.perf_counter()
        o = sharded(*dev_in)
        jax.block_until_ready(o)
        times.append(time.perf_counter() - t0)
    exec_ns = int(min(times) * 1e9) if times else None
    results = [
        {nm: np.asarray(out_arrs[i]).reshape(
            n_cores, *out_avals[i].shape)[c]
         for i, nm in enumerate(out_names)}
        for c in range(n_cores)]
    return results, exec_ns, times


# ---------------------------------------------------------------- entry
def kernel(**inputs):
    x = np.asarray(inputs["x"], np.float32)
    edge_index = np.asarray(inputs["edge_index"])
    Wi = np.asarray(inputs["Wi"], np.float32)
    bi = np.asarray(inputs["bi"], np.float32)
    w1 = np.asarray(inputs["w1"], np.float32)
    w2 = np.asarray(inputs["w2"], np.float32)
    gamma = np.asarray(inputs["gamma"], np.float32)
    beta = np.asarray(inputs["beta"], np.float32)
    Wo = np.asarray(inputs["Wo"], np.float32)
    bo = np.asarray(inputs["bo"], np.float32)

    n_nodes, fin = x.shape
    hid = Wi.shape[1]
    nlay = w1.shape[0]
    outd = Wo.shape[1]
    nblk = -(-n_nodes // (NC * BLK))
    shard = nblk * BLK
    npad = NC * shard

    assert npad % 4 == 0
    prep = _host_prep(edge_index, n_nodes, npad, shard, nblk, CHUNK_BLKS)
    ntiles = prep["ntiles"]
    max_chunk_elems = max(ch["nelem"] for ch in prep["chunks"])

    cfg = dict(fin=fin, hid=hid, outd=outd, nlay=nlay, shard=shard, nblk=nblk,
               npad=npad, n=n_nodes, ntiles=ntiles, chunks=prep["chunks"],
               gcols=prep["gcols"], max_chunk_elems=max_chunk_elems,
               debug=DEBUG)

    # host-side tensor prep
    xpad = np.zeros((npad, fin), np.float32)
    xpad[:n_nodes] = x
    w1p = (w1 + np.eye(hid, dtype=np.float32)).astype(np.float16)
    w2p = (w2 + ALPHA * np.eye(hid, dtype=np.float32)).astype(np.float16)
    gbv = np.zeros((hid, 2 * nlay), np.float32)
    for l in range(nlay):
        gbv[:, 2 * l] = gamma[l]
        gbv[:, 2 * l + 1] = beta[l]
    bv = np.zeros((hid, 2), np.float32)
    bv[:, 0] = bi
    bv[:outd, 1] = bo

    in_maps = []
    for c in range(NC):
        xs = xpad[c * shard:(c + 1) * shard].astype(np.float16)
        in_maps.append({
            "xT": np.ascontiguousarray(xs.T),
            "gidx": prep["gidx"][c],
            "edat": prep["edat"][c],
            "enrm": prep["enrm"][c],
            "wi": Wi.astype(np.float16),
            "w1p": w1p, "w2p": w2p, "gb": gbv,
            "wo": Wo.astype(np.float16), "bvec": bv,
            "selfn": prep["selfn"][c],
        })

    nc = bacc.Bacc("TRN2", target_bir_lowering=False, debug=False,
                   num_devices=NC, num_swdge_queues=2)
    _build(nc, cfg)
    nc.compile()
    if BUILD_ONLY:
        return nc, in_maps
    global LAST, EXEC_NS, EXEC_TIMES
    if TIME_REPEATS > 0:
        results, EXEC_NS, EXEC_TIMES = _run_spmd_timed(
            nc, in_maps, NC, TIME_REPEATS)

        class _R:
            pass
        res = _R()
        res.results = results
        res.exec_time_ns = EXEC_NS
        res.mean_exec_time_ns = None
        LAST = res
    else:
        res = run_bass_kernel_spmd(nc, in_maps, list(range(NC)),
                                   trace=TRACE, tmpdir=TRACE_DIR)
        LAST = res

    parts = []
    for c in range(NC):
        arr = res.results[c]["out"]                    # [outd, shard]
        parts.append(np.asarray(arr).T)                # [shard, outd]
    full = np.concatenate(parts, axis=0)[:n_nodes].astype(np.float32)
    return full + bo[None, :]



# revision 19
# speedup vs baseline: 1.8454x; 1.8454x over previous
"""GCN2 (GCNII) message-passing kernel for 8 Trainium2 NeuronCores.

Strategy (1D node sharding per the spec sharding_hint), v2 "transposed"
pipeline:
- Nodes padded to NPAD = 8*NBLK*128 and sharded contiguously across 8 cores.
- Per-node-feature state is kept feature-major: hT/h0T/zT are [64, shard]
  SBUF tiles, so BatchNorm affine + ReLU + bias become per-partition
  scale/bias ops on the (otherwise idle) Activation engine, and BN stats
  come for free from activation accum_out columns.
- Edges (self-loops excluded) are partitioned by destination core, sorted
  by (chunk of CHUNK_BLKS dest blocks, q=rr%4, dest block), padded to 128 per
  (chunk, q) bucket uniformly across cores (SPMD).
- The support table [NPAD, 64] f16 lives in DRAM with the partition-major
  row remap rr = c*shard + p*nblk + b; one dma_gather per chunk fetches a
  512-byte element (4 consecutive table rows, idx w = rr//4, int16) per
  edge; q selects the 64-col quarter at matmul time.
- Segment-sum on the PE: per 128-edge tile, DVE builds a one-hot
  P[e, d] = (iota==col_rel)*norm and PE accumulates
  aggT[feat, dest] += g_slice.T @ P into the dest block's [64,128] PSUM
  tile.  The same PSUM tile also accumulates initial = (w2+aI).T @ h0T
  and the self-loop term (dinv^2-scaled local support via identity
  matmul), so z = agg + initial never materializes through DVE.
- BN stats per core ride AllGather (cheaper than AllReduce); the support
  shards are AllGathered into the next layer's DRAM table.
- f16 table/gather/P/weights (validated ~7e-4 rel err); f32 PSUM/stats.
"""
import math
from contextlib import ExitStack

import numpy as np

import concourse.bass as bass
import concourse.bacc as bacc
import concourse.tile as tile
from concourse import mybir
from concourse.bass_utils import run_bass_kernel_spmd
from concourse.masks import make_identity

DT = mybir.dt
F16 = DT.float16
F32 = DT.float32
AF = mybir.ActivationFunctionType
OP = mybir.AluOpType

NC = 8
BLK = 128
ALPHA = 0.5
EPS = 1e-5
CHUNK_BLKS = 6
NVAR = 6           # max dest-block span of one edge tile
GRP = 4            # blocks per 512-col matmul group
DEBUG = False
BUILD_ONLY = False
GATHER_QUEUES = 2   # sim: set to 1 (sim forbids sem sharing across queues)
TIME_REPEATS = 0
EXEC_NS = None
EXEC_TIMES = None
TRACE = False
TRACE_DIR = None
LAST = None


# ---------------------------------------------------------------- host prep
def _host_prep(edge_index, n_nodes, npad, shard, nblk, chunk_blks):
    """Per-core edge arrays + SPMD-uniform gather/matmul schedule.

    Table rows use the partition-major remap rr = c*shard + p*nblk + b.
    dma_gather (int16 idx, 512B elems) reads the [npad//4, 4*64] wide view:
    idx w = rr//4 selects a 4-node group; q = rr%4 picks the 64-col quarter,
    chosen per tile (edges sorted by (chunk, q, block)).
    """
    e = np.asarray(edge_index)
    row = e[0].astype(np.int64)
    col = e[1].astype(np.int64)
    # degrees include the self loop (gcn_norm adds one per node)
    deg = np.bincount(col, minlength=n_nodes).astype(np.float64) + 1.0
    dinv = deg ** -0.5
    norm = (dinv[row] * dinv[col]).astype(np.float32)
    selfn_full = np.zeros(npad, np.float32)
    selfn_full[:n_nodes] = (dinv * dinv).astype(np.float32)

    core = col // shard
    blk = (col % shard) // BLK
    crel_g = col % shard

    c_src = row // shard
    r_src = row % shard
    rr = c_src * shard + (r_src % BLK) * nblk + (r_src // BLK)
    # 256B gather elements: the [npad, 64] f16 table viewed as
    # [npad//2, 128]; sub = table half (keeps idx < 32768 for int16),
    # w = row-pair within the half, q = which 64-col half of the pair.
    half = npad // 2
    sub_all = rr // half
    w_all = (rr % half) // 2
    q_all = rr % 2
    bq_all = sub_all * 2 + q_all

    chunks_b = []
    b0 = 0
    while b0 < nblk:
        chunks_b.append((b0, min(b0 + chunk_blks, nblk)))
        b0 = min(b0 + chunk_blks, nblk)
    nchunk = len(chunks_b)
    chunk_of = np.zeros(nblk, dtype=np.int64)
    for ci, (cb0, cb1) in enumerate(chunks_b):
        chunk_of[cb0:cb1] = ci
    key_chunk = chunk_of[blk]

    order = np.lexsort((blk, bq_all, key_chunk, core))
    S = dict(w=w_all[order], q=q_all[order], bq=bq_all[order], blk=blk[order],
             core=core[order], chunk=key_chunk[order], crel=crel_g[order],
             nrm=norm[order])

    cnt = np.zeros((NC, nchunk, 4), dtype=np.int64)
    np.add.at(cnt, (S["core"], S["chunk"], S["bq"]), 1)
    run_len = (-(-cnt.max(axis=0) // BLK) * BLK)        # [nchunk, 4]
    run_len = np.maximum(run_len, BLK)

    ntiles = int(run_len.sum()) // BLK
    tot = ntiles * BLK
    p_w = np.zeros((NC, tot), dtype=np.int16)
    p_crel = np.full((NC, tot), 20000.0, dtype=np.float32)
    p_nrm = np.zeros((NC, tot), dtype=np.float32)
    p_blk = np.full((NC, tot), -1, dtype=np.int64)

    run_off = np.zeros((nchunk, 4), dtype=np.int64)
    acc = 0
    for ci in range(nchunk):
        for q in range(4):
            run_off[ci, q] = acc
            acc += run_len[ci, q]

    grp_key = S["core"] * (nchunk * 4) + S["chunk"] * 4 + S["bq"]
    grp_change = np.concatenate([[True], grp_key[1:] != grp_key[:-1]])
    grp_start = np.where(grp_change)[0]
    rank = np.arange(len(grp_key)) - np.repeat(
        grp_start, np.diff(np.concatenate([grp_start, [len(grp_key)]])))
    slot = run_off[S["chunk"], S["bq"]] + rank
    p_w[S["core"], slot] = S["w"].astype(np.int16)
    p_crel[S["core"], slot] = S["crel"].astype(np.float32)
    p_nrm[S["core"], slot] = S["nrm"]
    p_blk[S["core"], slot] = S["blk"]

    # schedule: per chunk -> one call + tiles
    chunks = []
    gidx_cols = 0
    tile_global = 0
    first_tile = {}
    last_tile = {}
    for ci, (cb0, cb1) in enumerate(chunks_b):
        nidx = int(run_len[ci].sum())
        calls = []
        tiles = []
        erel = 0
        for s in range(2):
            base = int(run_off[ci, 2 * s])
            n_s = int(run_len[ci, 2 * s] + run_len[ci, 2 * s + 1])
            calls.append(dict(gi0=gidx_cols, num_idxs=n_s, base=base,
                              e0=erel, sub=s))
            gidx_cols += n_s // 16
            for qq in range(2):
                bq = 2 * s + qq
                off = int(run_off[ci, bq])
                nq = int(run_len[ci, bq])
                nt = nq // BLK
                for t in range(nt):
                    s0 = off + t * BLK
                    blks = p_blk[:, s0:s0 + BLK]
                    real = blks >= 0
                    if real.any():
                        b_lo = int(blks[real].min())
                        b_hi = int(blks[real].max())
                    else:
                        b_lo = b_hi = cb0
                    assert b_hi - b_lo < NVAR, "tile spans too many blocks"
                    td = dict(slot0=s0, erel=erel, q=qq, b_lo=b_lo,
                              tid=tile_global,
                              pairs=list(range(b_lo, b_hi + 1)))
                    for b in td["pairs"]:
                        if b not in first_tile:
                            first_tile[b] = tile_global
                        last_tile[b] = tile_global
                    tiles.append(td)
                    tile_global += 1
                    erel += 1
        chunks.append(dict(calls=calls, tiles=tiles, b0=cb0, b1=cb1,
                           nelem=nidx))
    assert tile_global == ntiles

    # one PSUM accumulation group per block-PAIR (2KB zero region):
    # stop=True goes on the last emitted matmul touching the pair.
    last_pair = {}
    for b, lt in last_tile.items():
        last_pair[b // 2] = max(last_pair.get(b // 2, -1), lt)
    for ch in chunks:
        for td in ch["tiles"]:
            stop_idx = {}
            for i, b in enumerate(td["pairs"]):
                if last_pair[b // 2] == td["tid"]:
                    stop_idx[b // 2] = i
            td["flags"] = [(b, b - td["b_lo"],
                            first_tile[b] == td["tid"],
                            stop_idx.get(b // 2) == i)
                           for i, b in enumerate(td["pairs"])]

    # edat: col_rel - b_lo*128 (f32); enrm: norm (f16) per tile slot
    edat = np.zeros((NC, BLK, ntiles), dtype=np.float32)
    enrm = np.zeros((NC, BLK, ntiles), dtype=np.float32)
    ti = 0
    for ch in chunks:
        for td in ch["tiles"]:
            s0 = td["slot0"]
            cr = p_crel[:, s0:s0 + BLK] - (td["b_lo"] * BLK)
            cr[p_blk[:, s0:s0 + BLK] < 0] = 20000.0
            edat[:, :, ti] = cr
            enrm[:, :, ti] = p_nrm[:, s0:s0 + BLK]
            ti += 1
    assert ti == ntiles

    # gidx: per call, 16-wrapped layout replicated across 128 partitions
    gidx = np.zeros((NC, BLK, gidx_cols), dtype=np.int16)
    for ch in chunks:
        for ca in ch["calls"]:
            base, n = ca["base"], ca["num_idxs"]
            vals = p_w[:, base:base + n]
            wrap = vals.reshape(NC, n // 16, 16).transpose(0, 2, 1)
            gi0 = ca["gi0"]
            for rep in range(8):
                gidx[:, rep * 16:(rep + 1) * 16, gi0:gi0 + n // 16] = wrap

    # per-core dinv^2 column layout [128, nblk]
    selfn = np.zeros((NC, BLK, nblk), dtype=np.float32)
    for c in range(NC):
        sl = selfn_full[c * shard:(c + 1) * shard].reshape(nblk, BLK)
        selfn[c] = sl.T
    return dict(gidx=gidx, edat=edat, enrm=enrm, chunks=chunks,
                ntiles=ntiles, gcols=gidx_cols, selfn=selfn)


# ---------------------------------------------------------------- program
def _build(nc, cfg):
    fin = cfg["fin"]
    hid = cfg["hid"]
    outd = cfg["outd"]
    nlay = cfg["nlay"]
    shard = cfg["shard"]
    nblk = cfg["nblk"]
    npad = cfg["npad"]
    n_nodes = cfg["n"]
    ntiles = cfg["ntiles"]

    xT = nc.declare_dram_parameter("xT", [fin, shard], F16, isOutput=False)
    gidx = nc.declare_dram_parameter("gidx", [BLK, cfg["gcols"]], DT.int16, isOutput=False)
    edat = nc.declare_dram_parameter("edat", [BLK, ntiles], F32, isOutput=False)
    enrm = nc.declare_dram_parameter("enrm", [BLK, ntiles], F32, isOutput=False)
    wi = nc.declare_dram_parameter("wi", [fin, hid], F16, isOutput=False)
    w1p = nc.declare_dram_parameter("w1p", [nlay, hid, hid], F16, isOutput=False)
    w2p = nc.declare_dram_parameter("w2p", [nlay, hid, hid], F16, isOutput=False)
    gb = nc.declare_dram_parameter("gb", [hid, 2 * nlay], F32, isOutput=False)
    wo = nc.declare_dram_parameter("wo", [hid, outd], F16, isOutput=False)
    bvec = nc.declare_dram_parameter("bvec", [hid, 2], F32, isOutput=False)
    selfn = nc.declare_dram_parameter("selfn", [BLK, nblk], F32, isOutput=False)
    out_p = nc.declare_dram_parameter("out", [outd, shard], F16, isOutput=True)
    dbg = cfg.get("debug", False)
    if dbg:
        dbg_h = nc.declare_dram_parameter("dbg_h", [hid, shard], F16, isOutput=True)
        dbg_sup = nc.declare_dram_parameter("dbg_sup", [BLK, nblk * hid], F16, isOutput=True)
        dbg_self = nc.declare_dram_parameter("dbg_self", [BLK, nblk * hid], F16, isOutput=True)
        dbg_z = nc.declare_dram_parameter("dbg_z", [hid, shard], F16, isOutput=True)
        dbg_st = nc.declare_dram_parameter("dbg_st", [hid, 2], F32, isOutput=True)
        dbg_sc = nc.declare_dram_parameter("dbg_sc", [hid, 2], F32, isOutput=True)

    core_ids = list(range(NC))
    inv_n = 1.0 / float(n_nodes)
    npair = (nblk + 1) // 2

    # node-column groups of GRP blocks (512 cols) for wide matmuls
    groups = []
    b0 = 0
    while b0 < nblk:
        b1 = min(b0 + GRP, nblk)
        groups.append((b0, b1))
        b0 = b1

    with tile.TileContext(nc) as tc, ExitStack() as ctx:
        const = ctx.enter_context(tc.tile_pool(name="const", bufs=1))
        dram = ctx.enter_context(tc.tile_pool(name="dram", bufs=1, space="DRAM"))

        tables = [dram.tile([npad, hid], F16, addr_space="Shared",
                            name=f"table{i}") for i in range(nlay)]
        sup_local = dram.tile([shard, hid], F16)
        stats_in = dram.tile([hid, 2], F32)
        stats_outs = [dram.tile([NC * hid, 2], F32, addr_space="Shared",
                                name=f"statso{i}") for i in range(nlay)]

        # ---- constants
        iotas = []
        for v in range(NVAR):
            iota_i = const.tile([BLK, BLK], DT.int16, tag="ioti")
            nc.gpsimd.iota(iota_i, pattern=[[1, BLK]], base=v * BLK,
                           channel_multiplier=0)
            iota_v = const.tile([BLK, BLK], F16, tag=f"iotf{v}")
            nc.vector.tensor_copy(iota_v, iota_i)
            iotas.append(iota_v)
        ident16 = const.tile([BLK, BLK], F16)
        make_identity(nc, ident16)

        wi_sb = const.tile([fin, hid], F16)
        nc.sync.dma_start(out=wi_sb, in_=wi[:, :])
        w1_sb = const.tile([hid, nlay * hid], F16)
        w2_sb = const.tile([hid, nlay * hid], F16)
        for l in range(nlay):
            nc.sync.dma_start(out=w1_sb[:, l * hid:(l + 1) * hid], in_=w1p[l, :, :])
            nc.sync.dma_start(out=w2_sb[:, l * hid:(l + 1) * hid], in_=w2p[l, :, :])
        wo_sb = const.tile([hid, outd], F16)
        nc.sync.dma_start(out=wo_sb, in_=wo[:, :])
        gb_sb = const.tile([hid, 2 * nlay], F32)
        nc.sync.dma_start(out=gb_sb, in_=gb[:, :])
        bvec_sb = const.tile([hid, 2], F32)
        nc.sync.dma_start(out=bvec_sb, in_=bvec[:, :])
        selfn_sb = const.tile([BLK, nblk], F32)
        nc.sync.dma_start(out=selfn_sb, in_=selfn[:, :])
        gidx_sb = const.tile([BLK, cfg["gcols"]], DT.int16)
        edat_sb = const.tile([BLK, ntiles], F32)
        enrm_sb = const.tile([BLK, ntiles], F32)

        # ---- persistent state (feature-major)
        hT = const.tile([hid, shard], F16)
        h0T = const.tile([hid, shard], F16)
        zT = const.tile([hid, shard], F16)
        supsend = const.tile([BLK, nblk * hid], F16)
        stats_pg = const.tile([hid, 2 * npair], F32)
        stats_col = const.tile([hid, 2], F32)
        statsg_sb = const.tile([hid, 2 * NC], F32)

        gpool = ctx.enter_context(tc.tile_pool(name="gpool", bufs=3))
        ppool = ctx.enter_context(tc.tile_pool(name="ppool", bufs=8))
        tpool = ctx.enter_context(tc.tile_pool(name="tpool", bufs=5))
        spool = ctx.enter_context(tc.tile_pool(name="spool", bufs=4))

        # ---- input layer emitted per-group inside layer 0's support loop

        # edge-schedule tables aren't needed until the first gather;
        # loading them here overlaps the input layer's compute
        nc.sync.dma_start(out=gidx_sb, in_=gidx[:, :])
        nc.sync.dma_start(out=edat_sb, in_=edat[:, :])
        nc.sync.dma_start(out=enrm_sb, in_=enrm[:, :])

        if dbg:
            nc.sync.dma_start(out=dbg_h[:, :], in_=hT)

        # ---- layers
        scl_p = sht_p = None
        for l in range(nlay):
            w1s = w1_sb[:, l * hid:(l + 1) * hid]
            w2s = w2_sb[:, l * hid:(l + 1) * hid]

            # (a) fused: hT += relu(scl*zT+sht) [layer l-1 BN], then
            #     supT = (w1+I).T @ hT; transpose to node-major; write table
            #     shard + dinv^2-scaled self-loop copy.  Copies on DVE (wide).
            lay_in = ExitStack()
            sup_ps = lay_in.enter_context(
                tc.tile_pool(name=f"sup{l}", bufs=2, space="PSUM"))
            tr_ps = lay_in.enter_context(
                tc.tile_pool(name=f"tr{l}", bufs=3, space="PSUM"))
            if l == 0:
                xpool = lay_in.enter_context(tc.tile_pool(name="xpool", bufs=3))
                in_ps = lay_in.enter_context(
                    tc.tile_pool(name="in_ps", bufs=2, space="PSUM"))
            if True:
                for (g0, g1) in groups:
                    w = (g1 - g0) * BLK
                    c0 = g0 * BLK
                    if l == 0:
                        xg = xpool.tile([fin, GRP * BLK], F16, tag="xg")
                        nc.sync.dma_start(out=xg[:, :w], in_=xT[:, c0:c0 + w])
                        ph = in_ps.tile([hid, GRP * BLK], F32, tag="ph")
                        nc.tensor.matmul(ph[:, :w], lhsT=wi_sb, rhs=xg[:, :w],
                                         start=True, stop=True)
                        nc.scalar.activation(out=hT[:, c0:c0 + w],
                                             in_=ph[:, :w], func=AF.Relu,
                                             bias=bvec_sb[:, 0:1], scale=1.0)
                        nc.vector.tensor_copy(h0T[:, c0:c0 + w],
                                              hT[:, c0:c0 + w])
                    if scl_p is not None:
                        rt = tpool.tile([hid, GRP * BLK], F16, tag="rt")
                        nc.scalar.activation(out=rt[:, :w], in_=zT[:, c0:c0 + w],
                                             func=AF.Relu, scale=scl_p[:, 0:1],
                                             bias=sht_p[:, 0:1])
                        nc.gpsimd.tensor_tensor(out=hT[:, c0:c0 + w],
                                                in0=hT[:, c0:c0 + w],
                                                in1=rt[:, :w], op=OP.add)
                    sp = sup_ps.tile([hid, GRP * BLK], F32, tag="sp")
                    nc.tensor.matmul(sp[:, :w], lhsT=w1s, rhs=hT[:, c0:c0 + w],
                                     start=True, stop=True)
                    spf = spool.tile([hid, GRP * BLK], F16, tag="spf")
                    if groups.index((g0, g1)) % 4 == 3:
                        nc.scalar.activation(out=spf[:, :w], in_=sp[:, :w],
                                             func=AF.Copy)
                    else:
                        nc.vector.tensor_copy(spf[:, :w], sp[:, :w])
                    tp = tr_ps.tile([BLK, GRP * hid], F16, tag="tp")
                    for b in range(g0, g1):
                        boff = (b - g0) * BLK
                        toff = (b - g0) * hid
                        nc.tensor.transpose(out=tp[:, toff:toff + hid],
                                            in_=spf[:, boff:boff + BLK],
                                            identity=ident16[:hid, :hid])
                    nc.vector.tensor_copy(
                        supsend[:, g0 * hid:g1 * hid],
                        tp[:, :(g1 - g0) * hid])
                    gi = groups.index((g0, g1))
                    if gi % 5 == 4 or g1 == nblk:
                        s0 = groups[gi - gi % 5][0]
                        nc.sync.dma_start(
                            out=sup_local[:, :].rearrange(
                                "(p b) f -> p (b f)", p=BLK)[:, s0 * hid:g1 * hid],
                            in_=supsend[:, s0 * hid:g1 * hid])
            lay_in.close()
            if dbg and l == 0:
                nc.sync.dma_start(out=dbg_sup[:, :], in_=supsend)
            nc.gpsimd.collective_compute(
                "AllGather", OP.bypass, replica_groups=[core_ids],
                ins=[sup_local.opt()], outs=[tables[l].opt()])

            # (b) edge aggregation: one [64, 4*128] PSUM bank per chunk
            lay_ctx = ExitStack()
            agg_ps = lay_ctx.enter_context(
                tc.tile_pool(name=f"agg{l}", bufs=CHUNK_BLKS,
                             space="PSUM"))
            tw = tables[l][:, :].rearrange("(w q) f -> w (q f)", q=2)
            whalf = cfg["npad"] // 4          # row-pairs per table half
            for ci, ch in enumerate(cfg["chunks"]):
                nel = ch["nelem"]
                cb0, cb1 = ch["b0"], ch["b1"]
                g = gpool.tile([BLK, cfg["max_chunk_elems"]], F16,
                               tag="g")
                for ca in ch["calls"]:
                    n_s = ca["num_idxs"]
                    c0 = ca["e0"] * BLK
                    nc.gpsimd.dma_gather(
                        out_ap=g[:, c0:c0 + n_s].rearrange(
                            "p (t f) -> p t f", f=BLK),
                        in_ap=tw[ca["sub"] * whalf:(ca["sub"] + 1) * whalf, :],
                        idxs_ap=gidx_sb[:, ca["gi0"]:ca["gi0"] + n_s // 16],
                        num_idxs=n_s,
                        num_idxs_reg=n_s,
                        elem_size=BLK,
                        elem_step=BLK,
                        single_packet=(n_s <= 1024),
                        queue_num=(2 * ci + ca["sub"]) % GATHER_QUEUES,
                    )
                # one PSUM bank holds a PAIR of dest blocks ([64, 256] f32)
                aggt = {}
                for b in range(cb0, cb1):
                    pr = b // 2
                    if pr not in aggt:
                        aggt[pr] = agg_ps.tile([hid, 2 * BLK], F32,
                                               tag="agg", name=f"agg_p{pr}")
                    sl = aggt[pr][:, (b % 2) * BLK:(b % 2 + 1) * BLK]
                    # initial = (w2+aI).T @ h0T block; start zeroes the whole
                    # 2KB pair bank, so only the first block of a pair starts
                    nc.tensor.matmul(sl, lhsT=w2s,
                                     rhs=h0T[:, b * BLK:(b + 1) * BLK],
                                     start=(b % 2 == 0), stop=False)
                    # self loop: local support against dinv^2-scaled diag
                    dg = ppool.tile([BLK, BLK], F16, tag="dg")
                    nc.vector.tensor_scalar(out=dg, in0=ident16,
                                            scalar1=selfn_sb[:, b:b + 1],
                                            scalar2=None, op0=OP.mult)
                    nc.tensor.matmul(sl,
                                     lhsT=supsend[:, b * hid:(b + 1) * hid],
                                     rhs=dg,
                                     start=False, stop=False)
                for td in ch["tiles"]:
                    ti = td["tid"]
                    gsl = g[:, td["erel"] * BLK + td["q"] * hid:
                            td["erel"] * BLK + td["q"] * hid + hid]
                    for (b, var, st, sp) in td["flags"]:
                        pt = ppool.tile([BLK, BLK], F16, tag="p")
                        nc.vector.tensor_scalar(
                            out=pt, in0=iotas[var],
                            scalar1=edat_sb[:, ti:ti + 1],
                            scalar2=enrm_sb[:, ti:ti + 1],
                            op0=OP.is_equal, op1=OP.mult)
                        nc.tensor.matmul(
                            aggt[b // 2][:, (b % 2) * BLK:(b % 2 + 1) * BLK],
                            lhsT=gsl, rhs=pt, start=False, stop=sp)
                for pr in range(cb0 // 2, (cb1 + 1) // 2):
                    # evict pair: z to SBUF (f16) + stats accums
                    nc.scalar.activation(
                        out=zT[:, pr * 2 * BLK:(pr + 1) * 2 * BLK],
                        in_=aggt[pr], func=AF.Copy,
                        accum_out=stats_pg[:, pr:pr + 1])
                    zq = tpool.tile([hid, 2 * BLK], F32, tag="zq")
                    nc.scalar.activation(
                        out=zq, in_=aggt[pr], func=AF.Square,
                        accum_out=stats_pg[:, npair + pr:npair + pr + 1])
            lay_ctx.close()

            if dbg and l == 0:
                nc.sync.dma_start(out=dbg_z[:, :], in_=zT)

            # (c) stats: local reduce + AllGather + global reduce
            scr = tpool.tile([hid, npair], F32, tag="scr")
            nc.scalar.activation(out=scr, in_=stats_pg[:, :npair], func=AF.Copy,
                                 accum_out=stats_col[:, 0:1])
            scr2 = tpool.tile([hid, npair], F32, tag="scr2")
            nc.scalar.activation(out=scr2, in_=stats_pg[:, npair:], func=AF.Copy,
                                 accum_out=stats_col[:, 1:2])
            nc.sync.dma_start(out=stats_in.opt(), in_=stats_col)
            nc.gpsimd.collective_compute(
                "AllGather", OP.bypass, replica_groups=[core_ids],
                ins=[stats_in.opt()], outs=[stats_outs[l].opt()])
            nc.sync.dma_start(
                out=statsg_sb[:, :].rearrange("p (c s) -> p c s", c=NC),
                in_=stats_outs[l][:, :].rearrange("(c p) s -> p c s", c=NC))
            sumz = tpool.tile([hid, 1], F32, tag="sumz")
            sumq = tpool.tile([hid, 1], F32, tag="sumq")
            dscr = tpool.tile([hid, 2 * NC], F32, tag="dscr")
            nc.scalar.activation(
                out=dscr[:, :NC],
                in_=statsg_sb[:, :].rearrange("p (c s) -> p s c", c=NC)[:, 0, :],
                func=AF.Copy, accum_out=sumz)
            nc.scalar.activation(
                out=dscr[:, NC:],
                in_=statsg_sb[:, :].rearrange("p (c s) -> p s c", c=NC)[:, 1, :],
                func=AF.Copy, accum_out=sumq)

            # (d) BN affine params (all [64,1] columns)
            mt = tpool.tile([hid, 1], F32, tag="mt")
            nc.vector.tensor_scalar(out=mt, in0=sumz, scalar1=inv_n,
                                    scalar2=None, op0=OP.mult)
            vt = tpool.tile([hid, 1], F32, tag="vt")
            nc.vector.tensor_scalar(out=vt, in0=sumq, scalar1=inv_n,
                                    scalar2=None, op0=OP.mult)
            mm = tpool.tile([hid, 1], F32, tag="mm")
            nc.vector.tensor_tensor(out=mm, in0=mt, in1=mt, op=OP.mult)
            nc.vector.tensor_tensor(out=vt, in0=vt, in1=mm, op=OP.subtract)
            nc.vector.tensor_scalar(out=vt, in0=vt, scalar1=EPS, scalar2=None,
                                    op0=OP.add)
            rstd = tpool.tile([hid, 1], F32, tag="rstd")
            nc.vector.reciprocal(rstd, vt)
            nc.scalar.activation(out=rstd, in_=rstd, func=AF.Sqrt)
            scl = const.tile([hid, 1], F32, tag=f"scl{l}")
            nc.vector.tensor_tensor(out=scl, in0=gb_sb[:, 2 * l:2 * l + 1],
                                    in1=rstd, op=OP.mult)
            sht = const.tile([hid, 1], F32, tag=f"sht{l}")
            nc.vector.tensor_tensor(out=sht, in0=mt, in1=scl, op=OP.mult)
            nc.vector.tensor_tensor(out=sht, in0=gb_sb[:, 2 * l + 1:2 * l + 2],
                                    in1=sht, op=OP.subtract)
            if dbg and l == 0:
                nc.sync.dma_start(out=dbg_st[:, :], in_=stats_col)
                dbg_sc_t = const.tile([hid, 2], F32, tag="dbg_sc_t")
                nc.vector.tensor_copy(dbg_sc_t[:, 0:1], scl)
                nc.vector.tensor_copy(dbg_sc_t[:, 1:2], sht)
                nc.sync.dma_start(out=dbg_sc[:, :], in_=dbg_sc_t)
            scl_p, sht_p = scl, sht

        # ---- output layer: hT += relu(scl*zT+sht); outT = Wo.T @ hT + bo
        with tc.tile_pool(name="out_ps", bufs=2, space="PSUM") as out_ps:
            for (g0, g1) in groups:
                w = (g1 - g0) * BLK
                c0 = g0 * BLK
                rt = tpool.tile([hid, GRP * BLK], F16, tag="rt")
                nc.scalar.activation(out=rt[:, :w], in_=zT[:, c0:c0 + w],
                                     func=AF.Relu, scale=scl_p[:, 0:1],
                                     bias=sht_p[:, 0:1])
                nc.vector.tensor_tensor(out=hT[:, c0:c0 + w],
                                        in0=hT[:, c0:c0 + w], in1=rt[:, :w],
                                        op=OP.add)
                po = out_ps.tile([outd, GRP * BLK], F32, tag="po")
                nc.tensor.matmul(po[:, :w], lhsT=wo_sb, rhs=hT[:, c0:c0 + w],
                                 start=True, stop=True)
                ost = tpool.tile([outd, GRP * BLK], F16, tag="ost")
                nc.vector.tensor_copy(ost[:, :w], po[:, :w])
                nc.sync.dma_start(out=out_p[:, c0:c0 + w], in_=ost[:, :w])
    return nc


# ------------------------------------------------------------- timed runner
def _run_spmd_timed(nc, in_maps, n_cores, repeats):
    """Mirror of bass2jax.run_bass_via_pjrt with repeat timing (no donation,
    inputs pre-staged on device)."""
    import jax
    import time
    from jax.sharding import Mesh, PartitionSpec, NamedSharding
    from jax.experimental.shard_map import shard_map
    from concourse import bass2jax

    bass2jax.install_neuronx_cc_hook()
    partition_name = (nc.partition_id_tensor.name
                     if nc.partition_id_tensor else None)
    in_names, out_names, out_avals, zero_outs = [], [], [], []
    for alloc in nc.m.functions[0].allocations:
        if not isinstance(alloc, mybir.MemoryLocationSet):
            continue
        name = alloc.memorylocations[0].name
        if alloc.kind == "ExternalInput":
            if name != partition_name:
                in_names.append(name)
        elif alloc.kind == "ExternalOutput":
            shape = tuple(alloc.tensor_shape)
            dtype = mybir.dt.np(alloc.dtype)
            out_names.append(name)
            out_avals.append(jax.core.ShapedArray(shape, dtype))
            zero_outs.append(np.zeros(shape, dtype))
    n_params = len(in_names)
    in_names_full = list(in_names) + list(out_names)
    if partition_name is not None:
        in_names_full.append(partition_name)

    def _body(*args):
        operands = list(args)
        if partition_name is not None:
            operands.append(bass2jax.partition_id_tensor())
        outs = bass2jax._bass_exec_p.bind(
            *operands, out_avals=tuple(out_avals),
            in_names=tuple(in_names_full), out_names=tuple(out_names),
            lowering_input_output_aliases=(),
            sim_require_finite=True, sim_require_nnan=True, nc=nc)
        return tuple(outs)

    devices = jax.devices()[:n_cores]
    mesh = Mesh(np.asarray(devices), ("core",))
    spec = PartitionSpec("core")
    n_outs = len(out_avals)
    sharded = jax.jit(shard_map(
        _body, mesh=mesh, in_specs=(spec,) * (n_params + n_outs),
        out_specs=(spec,) * n_outs, check_rep=False), keep_unused=True)
    concat_in = [
        np.concatenate([np.asarray(in_maps[c][nm]) for c in range(n_cores)],
                       axis=0)
        for nm in in_names]
    concat_zeros = [np.zeros((n_cores * z.shape[0], *z.shape[1:]), z.dtype)
                    for z in zero_outs]
    sh = NamedSharding(mesh, spec)
    dev_in = [jax.device_put(a, sh) for a in concat_in + concat_zeros]
    for a in dev_in:
        a.block_until_ready()
    out_arrs = sharded(*dev_in)
    jax.block_until_ready(out_arrs)
    times = []
    for _ in range(repeats):
        t0 = time.perf_counter()
        o = sharded(*dev_in)
        jax.block_until_ready(o)
        times.append(time.perf_counter() - t0)
    exec_ns = int(min(times) * 1e9) if times else None
    results = [
        {nm: np.asarray(out_arrs[i]).reshape(
            n_cores, *out_avals[i].shape)[c]
         for i, nm in enumerate(out_names)}
        for c in range(n_cores)]
    return results, exec_ns, times


# ---------------------------------------------------------------- entry
def kernel(**inputs):
    x = np.asarray(inputs["x"], np.float32)
    edge_index = np.asarray(inputs["edge_index"])
    Wi = np.asarray(inputs["Wi"], np.float32)
    bi = np.asarray(inputs["bi"], np.float32)
    w1 = np.asarray(inputs["w1"], np.float32)
    w2 = np.asarray(inputs["w2"], np.float32)
    gamma = np.asarray(inputs["gamma"], np.float32)
    beta = np.asarray(inputs["beta"], np.float32)
    Wo = np.asarray(inputs["Wo"], np.float32)
    bo = np.asarray(inputs["bo"], np.float32)

    n_nodes, fin = x.shape
    hid = Wi.shape[1]
    nlay = w1.shape[0]
    outd = Wo.shape[1]
    nblk = -(-n_nodes // (NC * BLK))
    shard = nblk * BLK
    npad = NC * shard

    assert npad % 4 == 0
    prep = _host_prep(edge_index, n_nodes, npad, shard, nblk, CHUNK_BLKS)
    ntiles = prep["ntiles"]
    max_chunk_elems = max(ch["nelem"] for ch in prep["chunks"])

    cfg = dict(fin=fin, hid=hid, outd=outd, nlay=nlay, shard=shard, nblk=nblk,
               npad=npad, n=n_nodes, ntiles=ntiles, chunks=prep["chunks"],
               gcols=prep["gcols"], max_chunk_elems=max_chunk_elems,
               debug=DEBUG)

    # host-side tensor prep
    xpad = np.zeros((npad, fin), np.float32)
    xpad[:n_nodes] = x
    w1p = (w1 + np.eye(hid, dtype=np.float32)).astype(np.float16)
    w2p = (w2 + ALPHA * np.eye(hid, dtype=np.float32)).astype(np.float16)
    gbv = np.zeros((hid, 2 * nlay), np.float32)
    for l in range(nlay):
        gbv[:, 2 * l] = gamma[l]
        gbv[:, 2 * l + 1] = beta[l]
    bv = np.zeros((hid, 2), np.float32)
    bv[:, 0] = bi
    bv[:outd, 1] = bo

    in_maps = []
    for c in range(NC):
        xs = xpad[c * shard:(c + 1) * shard].astype(np.float16)
        in_maps.append({
            "xT": np.ascontiguousarray(xs.T),
            "gidx": prep["gidx"][c],
            "edat": prep["edat"][c],
            "enrm": prep["enrm"][c],
            "wi": Wi.astype(np.float16),
            "w1p": w1p, "w2p": w2p, "gb": gbv,
            "wo": Wo.astype(np.float16), "bvec": bv,
            "selfn": prep["selfn"][c],
        })

    nc = bacc.Bacc("TRN2", target_bir_lowering=False, debug=False,
                   num_devices=NC, num_swdge_queues=GATHER_QUEUES)
    _build(nc, cfg)
    nc.compile()
    if BUILD_ONLY:
        return nc, in_maps
    global LAST, EXEC_NS, EXEC_TIMES
    if TIME_REPEATS > 0:
        results, EXEC_NS, EXEC_TIMES = _run_spmd_timed(
            nc, in_maps, NC, TIME_REPEATS)

        class _R:
            pass
        res = _R()
        res.results = results
        res.exec_time_ns = EXEC_NS
        res.mean_exec_time_ns = None
        LAST = res
    else:
        res = run_bass_kernel_spmd(nc, in_maps, list(range(NC)),
                                   trace=TRACE, tmpdir=TRACE_DIR)
        LAST = res

    parts = []
    for c in range(NC):
        arr = res.results[c]["out"]                    # [outd, shard]
        parts.append(np.asarray(arr).T)                # [shard, outd]
    full = np.concatenate(parts, axis=0)[:n_nodes].astype(np.float32)
    return full + bo[None, :]



# revision 24
# speedup vs baseline: 2.0248x; 1.0972x over previous
"""GCN2 (GCNII) message-passing kernel for 8 Trainium2 NeuronCores.

Strategy (1D node sharding per the spec sharding_hint), v2 "transposed"
pipeline:
- Nodes padded to NPAD = 8*NBLK*128 and sharded contiguously across 8 cores.
- Per-node-feature state is kept feature-major: hT/h0T/zT are [64, shard]
  SBUF tiles, so BatchNorm affine + ReLU + bias become per-partition
  scale/bias ops on the (otherwise idle) Activation engine, and BN stats
  come for free from activation accum_out columns.
- Edges (self-loops excluded) are partitioned by destination core, sorted
  by (chunk of CHUNK_BLKS dest blocks, q=rr%4, dest block), padded to 128 per
  (chunk, q) bucket uniformly across cores (SPMD).
- The support table [NPAD, 64] f16 lives in DRAM with the partition-major
  row remap rr = c*shard + p*nblk + b; one dma_gather per chunk fetches a
  512-byte element (4 consecutive table rows, idx w = rr//4, int16) per
  edge; q selects the 64-col quarter at matmul time.
- Segment-sum on the PE: per 128-edge tile, DVE builds a one-hot
  P[e, d] = (iota==col_rel)*norm and PE accumulates
  aggT[feat, dest] += g_slice.T @ P into the dest block's [64,128] PSUM
  tile.  The same PSUM tile also accumulates initial = (w2+aI).T @ h0T
  and the self-loop term (dinv^2-scaled local support via identity
  matmul), so z = agg + initial never materializes through DVE.
- BN stats per core ride AllGather (cheaper than AllReduce); the support
  shards are AllGathered into the next layer's DRAM table.
- f16 table/gather/P/weights (validated ~7e-4 rel err); f32 PSUM/stats.
"""
import math
from contextlib import ExitStack

import numpy as np

import concourse.bass as bass
import concourse.bacc as bacc
import concourse.tile as tile
from concourse import mybir
from concourse.bass_utils import run_bass_kernel_spmd
from concourse.masks import make_identity

DT = mybir.dt
F16 = DT.float16
F32 = DT.float32
AF = mybir.ActivationFunctionType
OP = mybir.AluOpType

NC = 8
BLK = 128
ALPHA = 0.5
EPS = 1e-5
CHUNK_BLKS = 6
NVAR = 6           # max dest-block span of one edge tile
GRP = 4            # blocks per 512-col matmul group
DEBUG = False
BUILD_ONLY = False
GATHER_QUEUES = 4   # sim: set to 1 (sim forbids sem sharing across queues)
TIME_REPEATS = 0
EXEC_NS = None
EXEC_TIMES = None
TRACE = False
TRACE_DIR = None
LAST = None


# ---------------------------------------------------------------- host prep
def _host_prep(edge_index, n_nodes, npad, shard, nblk, chunk_blks):
    """Per-core edge arrays + SPMD-uniform gather/matmul schedule.

    Table rows use the partition-major remap rr = c*shard + p*nblk + b.
    dma_gather (int16 idx, 512B elems) reads the [npad//4, 4*64] wide view:
    idx w = rr//4 selects a 4-node group; q = rr%4 picks the 64-col quarter,
    chosen per tile (edges sorted by (chunk, q, block)).
    """
    e = np.asarray(edge_index)
    row = e[0].astype(np.int64)
    col = e[1].astype(np.int64)
    # degrees include the self loop (gcn_norm adds one per node)
    deg = np.bincount(col, minlength=n_nodes).astype(np.float64) + 1.0
    dinv = deg ** -0.5
    norm = (dinv[row] * dinv[col]).astype(np.float32)
    selfn_full = np.zeros(npad, np.float32)
    selfn_full[:n_nodes] = (dinv * dinv).astype(np.float32)

    core = col // shard
    blk = (col % shard) // BLK
    crel_g = col % shard

    c_src = row // shard
    r_src = row % shard
    p_src = r_src % BLK
    b_src = r_src // BLK
    # 256B gather elements; the table is AllGathered as TWO halves split by
    # source PARTITION half (p<64 vs p>=64) so the second AG overlaps the
    # first half's gathers.  Within a half: row rr = c*(shard/2) +
    # (p%64)*nblk + b, gathered as row-pairs w = rr//2 (int16-safe),
    # q = rr%2 picks the 64-col half of the 256B element.
    sub_all = p_src // 64
    rr = c_src * (shard // 2) + (p_src % 64) * nblk + b_src
    w_all = rr // 2
    q_all = rr % 2
    bq_all = sub_all * 2 + q_all
    assert w_all.max() < 32768

    chunks_b = []
    b0 = 0
    while b0 < nblk:
        chunks_b.append((b0, min(b0 + chunk_blks, nblk)))
        b0 = min(b0 + chunk_blks, nblk)
    nchunk = len(chunks_b)
    chunk_of = np.zeros(nblk, dtype=np.int64)
    for ci, (cb0, cb1) in enumerate(chunks_b):
        chunk_of[cb0:cb1] = ci
    key_chunk = chunk_of[blk]

    order = np.lexsort((blk, bq_all, key_chunk, core))
    S = dict(w=w_all[order], q=q_all[order], bq=bq_all[order], blk=blk[order],
             core=core[order], chunk=key_chunk[order], crel=crel_g[order],
             nrm=norm[order])

    cnt = np.zeros((NC, nchunk, 4), dtype=np.int64)
    np.add.at(cnt, (S["core"], S["chunk"], S["bq"]), 1)
    run_len = (-(-cnt.max(axis=0) // BLK) * BLK)        # [nchunk, 4]
    run_len = np.maximum(run_len, BLK)

    ntiles = int(run_len.sum()) // BLK
    tot = ntiles * BLK
    p_w = np.zeros((NC, tot), dtype=np.int16)
    p_crel = np.full((NC, tot), 20000.0, dtype=np.float32)
    p_nrm = np.zeros((NC, tot), dtype=np.float32)
    p_blk = np.full((NC, tot), -1, dtype=np.int64)

    run_off = np.zeros((nchunk, 4), dtype=np.int64)
    acc = 0
    for ci in range(nchunk):
        for q in range(4):
            run_off[ci, q] = acc
            acc += run_len[ci, q]

    grp_key = S["core"] * (nchunk * 4) + S["chunk"] * 4 + S["bq"]
    grp_change = np.concatenate([[True], grp_key[1:] != grp_key[:-1]])
    grp_start = np.where(grp_change)[0]
    rank = np.arange(len(grp_key)) - np.repeat(
        grp_start, np.diff(np.concatenate([grp_start, [len(grp_key)]])))
    slot = run_off[S["chunk"], S["bq"]] + rank
    p_w[S["core"], slot] = S["w"].astype(np.int16)
    p_crel[S["core"], slot] = S["crel"].astype(np.float32)
    p_nrm[S["core"], slot] = S["nrm"]
    p_blk[S["core"], slot] = S["blk"]

    # schedule: per chunk -> one call + tiles
    chunks = []
    gidx_cols = 0
    tile_global = 0
    first_tile = {}
    last_tile = {}
    for ci, (cb0, cb1) in enumerate(chunks_b):
        nidx = int(run_len[ci].sum())
        calls = []
        tiles = []
        erel = 0
        for s in range(2):
            base = int(run_off[ci, 2 * s])
            n_s = int(run_len[ci, 2 * s] + run_len[ci, 2 * s + 1])
            calls.append(dict(gi0=gidx_cols, num_idxs=n_s, base=base,
                              e0=erel, sub=s))
            gidx_cols += n_s // 16
            for qq in range(2):
                bq = 2 * s + qq
                off = int(run_off[ci, bq])
                nq = int(run_len[ci, bq])
                nt = nq // BLK
                for t in range(nt):
                    s0 = off + t * BLK
                    blks = p_blk[:, s0:s0 + BLK]
                    real = blks >= 0
                    if real.any():
                        b_lo = int(blks[real].min())
                        b_hi = int(blks[real].max())
                    else:
                        b_lo = b_hi = cb0
                    assert b_hi - b_lo < NVAR, "tile spans too many blocks"
                    td = dict(slot0=s0, erel=erel, q=qq, b_lo=b_lo,
                              tid=tile_global,
                              pairs=list(range(b_lo, b_hi + 1)))
                    for b in td["pairs"]:
                        if b not in first_tile:
                            first_tile[b] = tile_global
                        last_tile[b] = tile_global
                    tiles.append(td)
                    tile_global += 1
                    erel += 1
        chunks.append(dict(calls=calls, tiles=tiles, b0=cb0, b1=cb1,
                           nelem=nidx))
    assert tile_global == ntiles

    # one PSUM accumulation group per block-PAIR (2KB zero region):
    # stop=True goes on the last emitted matmul touching the pair.
    last_pair = {}
    for b, lt in last_tile.items():
        last_pair[b // 2] = max(last_pair.get(b // 2, -1), lt)
    for ch in chunks:
        for td in ch["tiles"]:
            stop_idx = {}
            for i, b in enumerate(td["pairs"]):
                if last_pair[b // 2] == td["tid"]:
                    stop_idx[b // 2] = i
            td["flags"] = [(b, b - td["b_lo"],
                            first_tile[b] == td["tid"],
                            stop_idx.get(b // 2) == i)
                           for i, b in enumerate(td["pairs"])]

    # edat: col_rel - b_lo*128 (f32); enrm: norm (f16) per tile slot
    edat = np.zeros((NC, BLK, ntiles), dtype=np.float32)
    enrm = np.zeros((NC, BLK, ntiles), dtype=np.float32)
    ti = 0
    for ch in chunks:
        for td in ch["tiles"]:
            s0 = td["slot0"]
            cr = p_crel[:, s0:s0 + BLK] - (td["b_lo"] * BLK)
            cr[p_blk[:, s0:s0 + BLK] < 0] = 20000.0
            edat[:, :, ti] = cr
            enrm[:, :, ti] = p_nrm[:, s0:s0 + BLK]
            ti += 1
    assert ti == ntiles

    # gidx: per call, 16-wrapped layout replicated across 128 partitions
    gidx = np.zeros((NC, BLK, gidx_cols), dtype=np.int16)
    for ch in chunks:
        for ca in ch["calls"]:
            base, n = ca["base"], ca["num_idxs"]
            vals = p_w[:, base:base + n]
            wrap = vals.reshape(NC, n // 16, 16).transpose(0, 2, 1)
            gi0 = ca["gi0"]
            for rep in range(8):
                gidx[:, rep * 16:(rep + 1) * 16, gi0:gi0 + n // 16] = wrap

    # per-core dinv^2 column layout [128, nblk]
    selfn = np.zeros((NC, BLK, nblk), dtype=np.float32)
    for c in range(NC):
        sl = selfn_full[c * shard:(c + 1) * shard].reshape(nblk, BLK)
        selfn[c] = sl.T
    return dict(gidx=gidx, edat=edat, enrm=enrm, chunks=chunks,
                ntiles=ntiles, gcols=gidx_cols, selfn=selfn)


# ---------------------------------------------------------------- program
def _build(nc, cfg):
    fin = cfg["fin"]
    hid = cfg["hid"]
    outd = cfg["outd"]
    nlay = cfg["nlay"]
    shard = cfg["shard"]
    nblk = cfg["nblk"]
    npad = cfg["npad"]
    n_nodes = cfg["n"]
    ntiles = cfg["ntiles"]

    xT = nc.declare_dram_parameter("xT", [fin, shard], F16, isOutput=False)
    gidx = nc.declare_dram_parameter("gidx", [BLK, cfg["gcols"]], DT.int16, isOutput=False)
    edat = nc.declare_dram_parameter("edat", [BLK, ntiles], F32, isOutput=False)
    enrm = nc.declare_dram_parameter("enrm", [BLK, ntiles], F32, isOutput=False)
    wi = nc.declare_dram_parameter("wi", [fin, hid], F16, isOutput=False)
    w1p = nc.declare_dram_parameter("w1p", [nlay, hid, hid], F16, isOutput=False)
    w2p = nc.declare_dram_parameter("w2p", [nlay, hid, hid], F16, isOutput=False)
    gb = nc.declare_dram_parameter("gb", [hid, 2 * nlay], F32, isOutput=False)
    wo = nc.declare_dram_parameter("wo", [hid, outd], F16, isOutput=False)
    bvec = nc.declare_dram_parameter("bvec", [hid, 2], F32, isOutput=False)
    selfn = nc.declare_dram_parameter("selfn", [BLK, nblk], F32, isOutput=False)
    out_p = nc.declare_dram_parameter("out", [outd, shard], F16, isOutput=True)
    dbg = cfg.get("debug", False)
    if dbg:
        dbg_h = nc.declare_dram_parameter("dbg_h", [hid, shard], F16, isOutput=True)
        dbg_sup = nc.declare_dram_parameter("dbg_sup", [BLK, nblk * hid], F16, isOutput=True)
        dbg_self = nc.declare_dram_parameter("dbg_self", [BLK, nblk * hid], F16, isOutput=True)
        dbg_z = nc.declare_dram_parameter("dbg_z", [hid, shard], F16, isOutput=True)
        dbg_st = nc.declare_dram_parameter("dbg_st", [hid, 2], F32, isOutput=True)
        dbg_sc = nc.declare_dram_parameter("dbg_sc", [hid, 2], F32, isOutput=True)

    core_ids = list(range(NC))
    inv_n = 1.0 / float(n_nodes)
    npair = (nblk + 1) // 2

    # node-column groups of GRP blocks (512 cols) for wide matmuls
    groups = []
    b0 = 0
    while b0 < nblk:
        b1 = min(b0 + GRP, nblk)
        groups.append((b0, b1))
        b0 = b1

    with tile.TileContext(nc) as tc, ExitStack() as ctx:
        const = ctx.enter_context(tc.tile_pool(name="const", bufs=1))
        dram = ctx.enter_context(tc.tile_pool(name="dram", bufs=1, space="DRAM"))

        tablesA = [dram.tile([npad // 2, hid], F16, addr_space="Shared",
                             name=f"tableA{i}") for i in range(nlay)]
        tablesB = [dram.tile([npad // 2, hid], F16, addr_space="Shared",
                             name=f"tableB{i}") for i in range(nlay)]
        sup_localA = dram.tile([shard // 2, hid], F16)
        sup_localB = dram.tile([shard // 2, hid], F16)
        stats_in = dram.tile([hid, 2], F32)
        stats_outs = [dram.tile([NC * hid, 2], F32, addr_space="Shared",
                                name=f"statso{i}") for i in range(nlay)]

        # ---- constants
        iotas = []
        for v in range(NVAR):
            iota_i = const.tile([BLK, BLK], DT.int16, tag="ioti")
            nc.gpsimd.iota(iota_i, pattern=[[1, BLK]], base=v * BLK,
                           channel_multiplier=0)
            iota_v = const.tile([BLK, BLK], F16, tag=f"iotf{v}")
            nc.vector.tensor_copy(iota_v, iota_i)
            iotas.append(iota_v)
        ident16 = const.tile([BLK, BLK], F16)
        make_identity(nc, ident16)

        wi_sb = const.tile([fin, hid], F16)
        nc.sync.dma_start(out=wi_sb, in_=wi[:, :])
        w1_sb = const.tile([hid, nlay * hid], F16)
        w2_sb = const.tile([hid, nlay * hid], F16)
        for l in range(nlay):
            nc.sync.dma_start(out=w1_sb[:, l * hid:(l + 1) * hid], in_=w1p[l, :, :])
            nc.sync.dma_start(out=w2_sb[:, l * hid:(l + 1) * hid], in_=w2p[l, :, :])
        wo_sb = const.tile([hid, outd], F16)
        nc.sync.dma_start(out=wo_sb, in_=wo[:, :])
        gb_sb = const.tile([hid, 2 * nlay], F32)
        nc.sync.dma_start(out=gb_sb, in_=gb[:, :])
        bvec_sb = const.tile([hid, 2], F32)
        nc.sync.dma_start(out=bvec_sb, in_=bvec[:, :])
        selfn_sb = const.tile([BLK, nblk], F32)
        nc.sync.dma_start(out=selfn_sb, in_=selfn[:, :])
        gidx_sb = const.tile([BLK, cfg["gcols"]], DT.int16)
        edat_sb = const.tile([BLK, ntiles], F32)
        enrm_sb = const.tile([BLK, ntiles], F32)

        # ---- persistent state (feature-major)
        hT = const.tile([hid, shard], F16)
        h0T = const.tile([hid, shard], F16)
        zT = const.tile([hid, shard], F16)
        supsend = const.tile([BLK, nblk * hid], F16)
        stats_pg = const.tile([hid, 2 * npair], F32)
        stats_col = const.tile([hid, 2], F32)
        statsg_sb = const.tile([hid, 2 * NC], F32)

        gpool = ctx.enter_context(tc.tile_pool(name="gpool", bufs=3))
        ppool = ctx.enter_context(tc.tile_pool(name="ppool", bufs=8))
        tpool = ctx.enter_context(tc.tile_pool(name="tpool", bufs=5))
        spool = ctx.enter_context(tc.tile_pool(name="spool", bufs=4))

        # ---- input layer emitted per-group inside layer 0's support loop

        # edge-schedule tables aren't needed until the first gather;
        # loading them here overlaps the input layer's compute
        nc.sync.dma_start(out=gidx_sb, in_=gidx[:, :])
        nc.sync.dma_start(out=edat_sb, in_=edat[:, :])
        nc.sync.dma_start(out=enrm_sb, in_=enrm[:, :])

        if dbg:
            nc.sync.dma_start(out=dbg_h[:, :], in_=hT)

        # ---- layers
        scl_p = sht_p = None
        for l in range(nlay):
            w1s = w1_sb[:, l * hid:(l + 1) * hid]
            w2s = w2_sb[:, l * hid:(l + 1) * hid]

            # (a) fused: hT += relu(scl*zT+sht) [layer l-1 BN], then
            #     supT = (w1+I).T @ hT; transpose to node-major; write table
            #     shard + dinv^2-scaled self-loop copy.  Copies on DVE (wide).
            lay_in = ExitStack()
            sup_ps = lay_in.enter_context(
                tc.tile_pool(name=f"sup{l}", bufs=2, space="PSUM"))
            tr_ps = lay_in.enter_context(
                tc.tile_pool(name=f"tr{l}", bufs=3, space="PSUM"))
            if l == 0:
                xpool = lay_in.enter_context(tc.tile_pool(name="xpool", bufs=3))
                in_ps = lay_in.enter_context(
                    tc.tile_pool(name="in_ps", bufs=2, space="PSUM"))
            if True:
                for (g0, g1) in groups:
                    w = (g1 - g0) * BLK
                    c0 = g0 * BLK
                    if l == 0:
                        xg = xpool.tile([fin, GRP * BLK], F16, tag="xg")
                        nc.sync.dma_start(out=xg[:, :w], in_=xT[:, c0:c0 + w])
                        ph = in_ps.tile([hid, GRP * BLK], F32, tag="ph")
                        nc.tensor.matmul(ph[:, :w], lhsT=wi_sb, rhs=xg[:, :w],
                                         start=True, stop=True)
                        nc.scalar.activation(out=hT[:, c0:c0 + w],
                                             in_=ph[:, :w], func=AF.Relu,
                                             bias=bvec_sb[:, 0:1], scale=1.0)
                        nc.vector.tensor_copy(h0T[:, c0:c0 + w],
                                              hT[:, c0:c0 + w])
                    if scl_p is not None:
                        rt = tpool.tile([hid, GRP * BLK], F16, tag="rt")
                        nc.scalar.activation(out=rt[:, :w], in_=zT[:, c0:c0 + w],
                                             func=AF.Relu, scale=scl_p[:, 0:1],
                                             bias=sht_p[:, 0:1])
                        nc.gpsimd.tensor_tensor(out=hT[:, c0:c0 + w],
                                                in0=hT[:, c0:c0 + w],
                                                in1=rt[:, :w], op=OP.add)
                    sp = sup_ps.tile([hid, GRP * BLK], F32, tag="sp")
                    nc.tensor.matmul(sp[:, :w], lhsT=w1s, rhs=hT[:, c0:c0 + w],
                                     start=True, stop=True)
                    spf = spool.tile([hid, GRP * BLK], F16, tag="spf")
                    if groups.index((g0, g1)) % 4 == 3:
                        nc.scalar.activation(out=spf[:, :w], in_=sp[:, :w],
                                             func=AF.Copy)
                    else:
                        nc.vector.tensor_copy(spf[:, :w], sp[:, :w])
                    tp = tr_ps.tile([BLK, GRP * hid], F16, tag="tp")
                    for b in range(g0, g1):
                        boff = (b - g0) * BLK
                        toff = (b - g0) * hid
                        nc.tensor.transpose(out=tp[:, toff:toff + hid],
                                            in_=spf[:, boff:boff + BLK],
                                            identity=ident16[:hid, :hid])
                    nc.vector.tensor_copy(
                        supsend[:, g0 * hid:g1 * hid],
                        tp[:, :(g1 - g0) * hid])
                    gi = groups.index((g0, g1))
                    if gi % 5 == 4 or g1 == nblk:
                        s0 = groups[gi - gi % 5][0]
                        nc.sync.dma_start(
                            out=sup_localA[:, :].rearrange(
                                "(p b) f -> p (b f)", p=64)[:, s0 * hid:g1 * hid],
                            in_=supsend[0:64, s0 * hid:g1 * hid])
                        nc.sync.dma_start(
                            out=sup_localB[:, :].rearrange(
                                "(p b) f -> p (b f)", p=64)[:, s0 * hid:g1 * hid],
                            in_=supsend[64:BLK, s0 * hid:g1 * hid])
            lay_in.close()
            if dbg and l == 0:
                nc.sync.dma_start(out=dbg_sup[:, :], in_=supsend)
            nc.gpsimd.collective_compute(
                "AllGather", OP.bypass, replica_groups=[core_ids],
                ins=[sup_localA.opt()], outs=[tablesA[l].opt()])
            nc.gpsimd.collective_compute(
                "AllGather", OP.bypass, replica_groups=[core_ids],
                ins=[sup_localB.opt()], outs=[tablesB[l].opt()])

            # (b) edge aggregation: one [64, 4*128] PSUM bank per chunk
            lay_ctx = ExitStack()
            agg_ps = lay_ctx.enter_context(
                tc.tile_pool(name=f"agg{l}", bufs=CHUNK_BLKS,
                             space="PSUM"))
            tws = [t[l][:, :].rearrange("(w q) f -> w (q f)", q=2)
                   for t in (tablesA, tablesB)]
            for ci, ch in enumerate(cfg["chunks"]):
                nel = ch["nelem"]
                cb0, cb1 = ch["b0"], ch["b1"]
                g = gpool.tile([BLK, cfg["max_chunk_elems"]], F16,
                               tag="g")
                for ca in ch["calls"]:
                    n_s = ca["num_idxs"]
                    c0 = ca["e0"] * BLK
                    nc.gpsimd.dma_gather(
                        out_ap=g[:, c0:c0 + n_s].rearrange(
                            "p (t f) -> p t f", f=BLK),
                        in_ap=tws[ca["sub"]][:, :],
                        idxs_ap=gidx_sb[:, ca["gi0"]:ca["gi0"] + n_s // 16],
                        num_idxs=n_s,
                        num_idxs_reg=n_s,
                        elem_size=BLK,
                        elem_step=BLK,
                        single_packet=(n_s <= 1024),
                        queue_num=(2 * ci + ca["sub"]) % GATHER_QUEUES,
                    )
                # one PSUM bank holds a PAIR of dest blocks ([64, 256] f32)
                aggt = {}
                for b in range(cb0, cb1):
                    pr = b // 2
                    if pr not in aggt:
                        aggt[pr] = agg_ps.tile([hid, 2 * BLK], F32,
                                               tag="agg", name=f"agg_p{pr}")
                    sl = aggt[pr][:, (b % 2) * BLK:(b % 2 + 1) * BLK]
                    # initial = (w2+aI).T @ h0T block; start zeroes the whole
                    # 2KB pair bank, so only the first block of a pair starts
                    nc.tensor.matmul(sl, lhsT=w2s,
                                     rhs=h0T[:, b * BLK:(b + 1) * BLK],
                                     start=(b % 2 == 0), stop=False)
                    # self loop: local support against dinv^2-scaled diag
                    dg = ppool.tile([BLK, BLK], F16, tag="dg")
                    nc.vector.tensor_scalar(out=dg, in0=ident16,
                                            scalar1=selfn_sb[:, b:b + 1],
                                            scalar2=None, op0=OP.mult)
                    nc.tensor.matmul(sl,
                                     lhsT=supsend[:, b * hid:(b + 1) * hid],
                                     rhs=dg,
                                     start=False, stop=False)
                for td in ch["tiles"]:
                    ti = td["tid"]
                    gsl = g[:, td["erel"] * BLK + td["q"] * hid:
                            td["erel"] * BLK + td["q"] * hid + hid]
                    for (b, var, st, sp) in td["flags"]:
                        pt = ppool.tile([BLK, BLK], F16, tag="p")
                        nc.vector.tensor_scalar(
                            out=pt, in0=iotas[var],
                            scalar1=edat_sb[:, ti:ti + 1],
                            scalar2=enrm_sb[:, ti:ti + 1],
                            op0=OP.is_equal, op1=OP.mult)
                        nc.tensor.matmul(
                            aggt[b // 2][:, (b % 2) * BLK:(b % 2 + 1) * BLK],
                            lhsT=gsl, rhs=pt, start=False, stop=sp)
                for pr in range(cb0 // 2, (cb1 + 1) // 2):
                    # evict pair: z to SBUF (f16) + stats accums
                    nc.scalar.activation(
                        out=zT[:, pr * 2 * BLK:(pr + 1) * 2 * BLK],
                        in_=aggt[pr], func=AF.Copy,
                        accum_out=stats_pg[:, pr:pr + 1])
                    zq = tpool.tile([hid, 2 * BLK], F32, tag="zq")
                    nc.scalar.activation(
                        out=zq, in_=aggt[pr], func=AF.Square,
                        accum_out=stats_pg[:, npair + pr:npair + pr + 1])
            lay_ctx.close()

            if dbg and l == 0:
                nc.sync.dma_start(out=dbg_z[:, :], in_=zT)

            # (c) stats: local reduce + AllGather + global reduce
            scr = tpool.tile([hid, npair], F32, tag="scr")
            nc.scalar.activation(out=scr, in_=stats_pg[:, :npair], func=AF.Copy,
                                 accum_out=stats_col[:, 0:1])
            scr2 = tpool.tile([hid, npair], F32, tag="scr2")
            nc.scalar.activation(out=scr2, in_=stats_pg[:, npair:], func=AF.Copy,
                                 accum_out=stats_col[:, 1:2])
            nc.sync.dma_start(out=stats_in.opt(), in_=stats_col)
            nc.gpsimd.collective_compute(
                "AllGather", OP.bypass, replica_groups=[core_ids],
                ins=[stats_in.opt()], outs=[stats_outs[l].opt()])
            nc.sync.dma_start(
                out=statsg_sb[:, :].rearrange("p (c s) -> p c s", c=NC),
                in_=stats_outs[l][:, :].rearrange("(c p) s -> p c s", c=NC))
            sumz = tpool.tile([hid, 1], F32, tag="sumz")
            sumq = tpool.tile([hid, 1], F32, tag="sumq")
            dscr = tpool.tile([hid, 2 * NC], F32, tag="dscr")
            nc.scalar.activation(
                out=dscr[:, :NC],
                in_=statsg_sb[:, :].rearrange("p (c s) -> p s c", c=NC)[:, 0, :],
                func=AF.Copy, accum_out=sumz)
            nc.scalar.activation(
                out=dscr[:, NC:],
                in_=statsg_sb[:, :].rearrange("p (c s) -> p s c", c=NC)[:, 1, :],
                func=AF.Copy, accum_out=sumq)

            # (d) BN affine params (all [64,1] columns)
            mt = tpool.tile([hid, 1], F32, tag="mt")
            nc.vector.tensor_scalar(out=mt, in0=sumz, scalar1=inv_n,
                                    scalar2=None, op0=OP.mult)
            vt = tpool.tile([hid, 1], F32, tag="vt")
            nc.vector.tensor_scalar(out=vt, in0=sumq, scalar1=inv_n,
                                    scalar2=None, op0=OP.mult)
            mm = tpool.tile([hid, 1], F32, tag="mm")
            nc.vector.tensor_tensor(out=mm, in0=mt, in1=mt, op=OP.mult)
            nc.vector.tensor_tensor(out=vt, in0=vt, in1=mm, op=OP.subtract)
            nc.vector.tensor_scalar(out=vt, in0=vt, scalar1=EPS, scalar2=None,
                                    op0=OP.add)
            rstd = tpool.tile([hid, 1], F32, tag="rstd")
            nc.vector.reciprocal(rstd, vt)
            nc.scalar.activation(out=rstd, in_=rstd, func=AF.Sqrt)
            scl = const.tile([hid, 1], F32, tag=f"scl{l}")
            nc.vector.tensor_tensor(out=scl, in0=gb_sb[:, 2 * l:2 * l + 1],
                                    in1=rstd, op=OP.mult)
            sht = const.tile([hid, 1], F32, tag=f"sht{l}")
            nc.vector.tensor_tensor(out=sht, in0=mt, in1=scl, op=OP.mult)
            nc.vector.tensor_tensor(out=sht, in0=gb_sb[:, 2 * l + 1:2 * l + 2],
                                    in1=sht, op=OP.subtract)
            if dbg and l == 0:
                nc.sync.dma_start(out=dbg_st[:, :], in_=stats_col)
                dbg_sc_t = const.tile([hid, 2], F32, tag="dbg_sc_t")
                nc.vector.tensor_copy(dbg_sc_t[:, 0:1], scl)
                nc.vector.tensor_copy(dbg_sc_t[:, 1:2], sht)
                nc.sync.dma_start(out=dbg_sc[:, :], in_=dbg_sc_t)
            scl_p, sht_p = scl, sht

        # ---- output layer: hT += relu(scl*zT+sht); outT = Wo.T @ hT + bo
        with tc.tile_pool(name="out_ps", bufs=2, space="PSUM") as out_ps:
            for (g0, g1) in groups:
                w = (g1 - g0) * BLK
                c0 = g0 * BLK
                rt = tpool.tile([hid, GRP * BLK], F16, tag="rt")
                nc.scalar.activation(out=rt[:, :w], in_=zT[:, c0:c0 + w],
                                     func=AF.Relu, scale=scl_p[:, 0:1],
                                     bias=sht_p[:, 0:1])
                nc.vector.tensor_tensor(out=hT[:, c0:c0 + w],
                                        in0=hT[:, c0:c0 + w], in1=rt[:, :w],
                                        op=OP.add)
                po = out_ps.tile([outd, GRP * BLK], F32, tag="po")
                nc.tensor.matmul(po[:, :w], lhsT=wo_sb, rhs=hT[:, c0:c0 + w],
                                 start=True, stop=True)
                ost = tpool.tile([outd, GRP * BLK], F16, tag="ost")
                nc.vector.tensor_copy(ost[:, :w], po[:, :w])
                nc.sync.dma_start(out=out_p[:, c0:c0 + w], in_=ost[:, :w])
    return nc


# ------------------------------------------------------------- timed runner
def _run_spmd_timed(nc, in_maps, n_cores, repeats):
    """Mirror of bass2jax.run_bass_via_pjrt with repeat timing (no donation,
    inputs pre-staged on device)."""
    import jax
    import time
    from jax.sharding import Mesh, PartitionSpec, NamedSharding
    from jax.experimental.shard_map import shard_map
    from concourse import bass2jax

    bass2jax.install_neuronx_cc_hook()
    partition_name = (nc.partition_id_tensor.name
                     if nc.partition_id_tensor else None)
    in_names, out_names, out_avals, zero_outs = [], [], [], []
    for alloc in nc.m.functions[0].allocations:
        if not isinstance(alloc, mybir.MemoryLocationSet):
            continue
        name = alloc.memorylocations[0].name
        if alloc.kind == "ExternalInput":
            if name != partition_name:
                in_names.append(name)
        elif alloc.kind == "ExternalOutput":
            shape = tuple(alloc.tensor_shape)
            dtype = mybir.dt.np(alloc.dtype)
            out_names.append(name)
            out_avals.append(jax.core.ShapedArray(shape, dtype))
            zero_outs.append(np.zeros(shape, dtype))
    n_params = len(in_names)
    in_names_full = list(in_names) + list(out_names)
    if partition_name is not None:
        in_names_full.append(partition_name)

    def _body(*args):
        operands = list(args)
        if partition_name is not None:
            operands.append(bass2jax.partition_id_tensor())
        outs = bass2jax._bass_exec_p.bind(
            *operands, out_avals=tuple(out_avals),
            in_names=tuple(in_names_full), out_names=tuple(out_names),
            lowering_input_output_aliases=(),
            sim_require_finite=True, sim_require_nnan=True, nc=nc)
        return tuple(outs)

    devices = jax.devices()[:n_cores]
    mesh = Mesh(np.asarray(devices), ("core",))
    spec = PartitionSpec("core")
    n_outs = len(out_avals)
    sharded = jax.jit(shard_map(
        _body, mesh=mesh, in_specs=(spec,) * (n_params + n_outs),
        out_specs=(spec,) * n_outs, check_rep=False), keep_unused=True)
    concat_in = [
        np.concatenate([np.asarray(in_maps[c][nm]) for c in range(n_cores)],
                       axis=0)
        for nm in in_names]
    concat_zeros = [np.zeros((n_cores * z.shape[0], *z.shape[1:]), z.dtype)
                    for z in zero_outs]
    sh = NamedSharding(mesh, spec)
    dev_in = [jax.device_put(a, sh) for a in concat_in + concat_zeros]
    for a in dev_in:
        a.block_until_ready()
    out_arrs = sharded(*dev_in)
    jax.block_until_ready(out_arrs)
    times = []
    for _ in range(repeats):
        t0 = time.perf_counter()
        o = sharded(*dev_in)
        jax.block_until_ready(o)
        times.append(time.perf_counter() - t0)
    exec_ns = int(min(times) * 1e9) if times else None
    results = [
        {nm: np.asarray(out_arrs[i]).reshape(
            n_cores, *out_avals[i].shape)[c]
         for i, nm in enumerate(out_names)}
        for c in range(n_cores)]
    return results, exec_ns, times


# ---------------------------------------------------------------- entry
def kernel(**inputs):
    x = np.asarray(inputs["x"], np.float32)
    edge_index = np.asarray(inputs["edge_index"])
    Wi = np.asarray(inputs["Wi"], np.float32)
    bi = np.asarray(inputs["bi"], np.float32)
    w1 = np.asarray(inputs["w1"], np.float32)
    w2 = np.asarray(inputs["w2"], np.float32)
    gamma = np.asarray(inputs["gamma"], np.float32)
    beta = np.asarray(inputs["beta"], np.float32)
    Wo = np.asarray(inputs["Wo"], np.float32)
    bo = np.asarray(inputs["bo"], np.float32)

    n_nodes, fin = x.shape
    hid = Wi.shape[1]
    nlay = w1.shape[0]
    outd = Wo.shape[1]
    nblk = -(-n_nodes // (NC * BLK))
    shard = nblk * BLK
    npad = NC * shard

    assert npad % 4 == 0
    prep = _host_prep(edge_index, n_nodes, npad, shard, nblk, CHUNK_BLKS)
    ntiles = prep["ntiles"]
    max_chunk_elems = max(ch["nelem"] for ch in prep["chunks"])

    cfg = dict(fin=fin, hid=hid, outd=outd, nlay=nlay, shard=shard, nblk=nblk,
               npad=npad, n=n_nodes, ntiles=ntiles, chunks=prep["chunks"],
               gcols=prep["gcols"], max_chunk_elems=max_chunk_elems,
               debug=DEBUG)

    # host-side tensor prep
    xpad = np.zeros((npad, fin), np.float32)
    xpad[:n_nodes] = x
    w1p = (w1 + np.eye(hid, dtype=np.float32)).astype(np.float16)
    w2p = (w2 + ALPHA * np.eye(hid, dtype=np.float32)).astype(np.float16)
    gbv = np.zeros((hid, 2 * nlay), np.float32)
    for l in range(nlay):
        gbv[:, 2 * l] = gamma[l]
        gbv[:, 2 * l + 1] = beta[l]
    bv = np.zeros((hid, 2), np.float32)
    bv[:, 0] = bi
    bv[:outd, 1] = bo

    in_maps = []
    for c in range(NC):
        xs = xpad[c * shard:(c + 1) * shard].astype(np.float16)
        in_maps.append({
            "xT": np.ascontiguousarray(xs.T),
            "gidx": prep["gidx"][c],
            "edat": prep["edat"][c],
            "enrm": prep["enrm"][c],
            "wi": Wi.astype(np.float16),
            "w1p": w1p, "w2p": w2p, "gb": gbv,
            "wo": Wo.astype(np.float16), "bvec": bv,
            "selfn": prep["selfn"][c],
        })

    nc = bacc.Bacc("TRN2", target_bir_lowering=False, debug=False,
                   num_devices=NC, num_swdge_queues=GATHER_QUEUES)
    _build(nc, cfg)
    nc.compile()
    if BUILD_ONLY:
        return nc, in_maps
    global LAST, EXEC_NS, EXEC_TIMES
    if TIME_REPEATS > 0:
        results, EXEC_NS, EXEC_TIMES = _run_spmd_timed(
            nc, in_maps, NC, TIME_REPEATS)

        class _R:
            pass
        res = _R()
        res.results = results
        res.exec_time_ns = EXEC_NS
        res.mean_exec_time_ns = None
        LAST = res
    else:
        res = run_bass_kernel_spmd(nc, in_maps, list(range(NC)),
                                   trace=TRACE, tmpdir=TRACE_DIR)
        LAST = res

    parts = []
    for c in range(NC):
        arr = res.results[c]["out"]                    # [outd, shard]
        parts.append(np.asarray(arr).T)                # [shard, outd]
    full = np.concatenate(parts, axis=0)[:n_nodes].astype(np.float32)
    return full + bo[None, :]

